# revision 7
# baseline (speedup 1.0000x reference)
"""BitLinear (layernorm -> absmax sign-quant -> sign-weight matmul -> bias*beta)
for Trainium2, batch-sharded across 8 NeuronCores.

Math (per row b, feature i, output o):
    mean_b  = mean(x[b,:]);  var_b = var(x[b,:])
    c_b     = max_i |x[b,i] - mean_b| * rsqrt(var_b + eps)
    A[b,i]  = sign(x[b,i] - mean_b)           (sign(xn) == sign(x - mean))
    out[b,o]= (c_b * sum_i A[b,i]*sign(W[o,i]) + bias[o]) * beta[o]

Fast path (gamma == 1): weight signs are computed on the host and shipped as
fp8e4 (+-1 exact), x is shipped as fp16 in both natural and transposed
layouts (sign flips from fp16 rounding cost ~6e-3 rel err, under the 2e-2
gate), the big GEMM runs fp8 DoubleRow (2 MACs/cell/cycle), absmax comes from
streaming max/min of raw x (amax = max(max-mean, mean-min)), and the output
is stored fp16 and upcast on the host. Each core handles 1024 batch rows; no
collectives. The general path (gamma != 1) keeps the slower bf16 pipeline.
"""
import sys

sys.path.insert(0, "/opt/trn_rl_repo")

from contextlib import ExitStack

import numpy as np

import concourse.bass as bass
import concourse.tile as tile
from concourse import mybir
from concourse.bass_utils import run_bass_kernel_spmd
from concourse.vector_clock import ScopedClock, VectorClock

N_CORES = 8
EPS = 1e-5
P = 128


# ---------------------------------------------------------------------------
# Workaround: this walrus build rejects CTRL instructions (Drain/NoOp) with
# more than one sync wait. Tile's final drain carries one wait per live
# processor. Split them across single-wait SP nops; SP program order makes
# this equivalent.
def _patched_drain_and_barrier(self, tick_clock, wait_clock):
    gc = tick_clock.global_clock
    for scope, vclock in ScopedClock({None: gc}).items():
        n = len(vclock)
        for i in range(n):
            if vclock[i] > 0:
                vec = [0] * n
                vec[i] = vclock[i]
                nop_inst = self.nc.sync.nop(nofuse=True, hint="split_drain_wait")
                wait_clock.add_sem_waits(
                    nop_inst.ins, ScopedClock({scope: VectorClock(vec)})
                )
    self.nc.sync.drain()
    self.nc.all_engine_barrier()
    assert self.sems is not None
    popped = self.nc._tile_sem_poison_stack.pop()
    assert popped is self._sem_poison
    self.nc.clear_and_free_semaphores(list(self.sems.allocated().values()))
    self.nc.all_engine_barrier()


tile.TileContext._drain_and_barrier = _patched_drain_and_barrier


# This walrus build allows at most ONE sync wait on ANY instruction. Tile's
# wait-assignment emits up to 4. Post-process the serialized BIR: move all but
# the last wait of each instruction onto same-engine NoOps placed just before
# it (engine program order preserves semantics; for DMAs this gates descriptor
# submission, which is strictly more conservative).
def _split_multi_waits(m: dict) -> dict:
    for fn in m["functions"]:
        for bb in fn["blocks"]:
            out = []
            for ins in bb["instructions"]:
                si = ins.get("sync_info") or {}
                waits = si.get("on_wait") or []
                if len(waits) > 1:
                    for i, w in enumerate(waits[:-1]):
                        out.append(
                            {
                                "debug": ins.get("debug", 0),
                                "engine": ins["engine"],
                                "ins": [],
                                "outs": [],
                                "name": f"{ins['name']}-w{i}",
                                "opcode": "NoOp",
                                "sync_info": {"on_update": [], "on_wait": [w]},
                                "text_hint": "split_wait",
                            }
                        )
                    si["on_wait"] = [waits[-1]]
                out.append(ins)
            bb["instructions"] = out
    return m


_orig_to_json_bytes = bass.Bass.to_json_bytes


def _patched_to_json_bytes(self):
    import orjson

    m = orjson.loads(_orig_to_json_bytes(self))
    return orjson.dumps(_split_multi_waits(m))


bass.Bass.to_json_bytes = _patched_to_json_bytes
# ---------------------------------------------------------------------------


def build_fast_program(b_c, d_in, d_out):
    """fp8 DoubleRow fast path (gamma == 1, any beta/bias). fp16 x, fp16 out."""
    KT = d_in // P  # contraction tiles (32)
    OG = d_out // P  # output-feature tiles (32)
    NB = 512  # matmul moving free dim = one PSUM bank of fp32
    BC = b_c // NB  # batch chunks (2)
    TPC = NB // P  # batch tiles per chunk (4)
    SC = 512  # bn_stats hardware max free size
    XS = 1024  # x natural streaming subchunk (free elems)
    NQ = d_in // XS  # subchunks per batch tile (4)
    G = 4  # k-tiles per transposed-input DMA
    GSZ = 16  # ogs per matmul group (w tiles resident per group)

    f32 = mybir.dt.float32
    f16 = mybir.dt.float16
    fp8 = mybir.dt.float8e4
    X = mybir.AxisListType.X
    A = mybir.AluOpType
    AF = mybir.ActivationFunctionType

    nc = bass.Bass("TRN2", target_bir_lowering=False, debug=False)
    x16 = nc.dram_tensor("x16", [b_c, d_in], f16, kind="ExternalInput")
    # host-prechunked transpose: xTc[h, p, kt, j] = x[h*NB + j, kt*128 + p]
    xTc = nc.dram_tensor("xTc", [BC, P, KT, NB], f16, kind="ExternalInput")
    # host-pretiled weight signs: wS[og, p, kt, oc] = sign(W[og*128+oc, kt*128+p])
    wS = nc.dram_tensor("wS", [OG, P, KT, P], fp8, kind="ExternalInput")
    bias = nc.dram_tensor("bias", [d_out], f32, kind="ExternalInput")
    beta = nc.dram_tensor("beta", [d_out], f32, kind="ExternalInput")
    outT = nc.dram_tensor("outT", [d_out, b_c], f16, kind="ExternalOutput")
    # per-batch-chunk scratches so a chunk's broadcast only depends on that
    # chunk's stats tiles (Tile tracks DRAM deps per tensor).
    mean_ds = [nc.dram_tensor(f"mean_d{h}", [NB], f16) for h in range(BC)]
    c_ds = [nc.dram_tensor(f"c_d{h}", [NB], f32) for h in range(BC)]

    with tile.TileContext(nc) as tc, ExitStack() as ctx:
        consts = ctx.enter_context(tc.tile_pool(name="consts", bufs=1))
        # 16 xs tiles are pre-issued in the prologue and consumed later, so
        # the pool must hold all of them plus rotation slack for chunk 1.
        xs_p = ctx.enter_context(tc.tile_pool(name="xs", bufs=20))
        small_p = ctx.enter_context(tc.tile_pool(name="small", bufs=4))
        a_p = ctx.enter_context(tc.tile_pool(name="a", bufs=1))
        xt_p = ctx.enter_context(tc.tile_pool(name="xt", bufs=3))
        w_p = ctx.enter_context(tc.tile_pool(name="w", bufs=GSZ + 2))
        ep_p = ctx.enter_context(tc.tile_pool(name="ep", bufs=4))
        ps_p = ctx.enter_context(tc.tile_pool(name="ps", bufs=2 * BC, space="PSUM"))

        # --- constants ---------------------------------------------------
        eps_t = consts.tile([P, 1], f32)
        nc.vector.memset(eps_t, EPS)
        # column j of these holds v[j*128 : (j+1)*128] (per-partition scalars)
        bias_t = consts.tile([P, OG], f32)
        nc.sync.dma_start(
            out=bias_t, in_=bass.AP(tensor=bias, offset=0, ap=[[1, P], [P, OG]])
        )
        beta_t = consts.tile([P, OG], f32)
        nc.sync.dma_start(
            out=beta_t, in_=bass.AP(tensor=beta, offset=0, ap=[[1, P], [P, OG]])
        )
        bb_t = consts.tile([P, OG], f32)
        nc.vector.tensor_mul(bb_t, bias_t, beta_t)

        # --- prologue: issue chunk-0 input loads + first transposed groups
        # + first weight columns so the DMA queue heads carry the critical
        # path (HBM is ~360 GB/s/core and the early window is oversubscribed).
        x_pre = {}  # (bt, q) -> tile
        for bt in range(TPC):
            for q in range(NQ):
                xt_ = xs_p.tile([P, XS], f16, tag="xs", name=f"xs{bt}_{q}")
                nc.sync.dma_start(
                    out=xt_, in_=x16[bt * P : (bt + 1) * P, q * XS : (q + 1) * XS]
                )
                x_pre[(bt, q)] = xt_

        xtg_pre = {}
        for gi in range(2):
            xtg = xt_p.tile([P, G, NB], f16, tag="xtg", name=f"xtgp{gi}")
            nc.sync.dma_start(
                out=xtg,
                in_=bass.AP(
                    tensor=xTc, offset=gi * G * NB, ap=[[KT * NB, P], [1, G * NB]]
                ),
            )
            xtg_pre[gi] = xtg

        w_tiles = {}
        for og in range(4):
            wt = w_p.tile([P, KT * P], fp8, tag="w", name=f"w{og}")
            nc.sync.dma_start(
                out=wt,
                in_=bass.AP(
                    tensor=wS, offset=og * P * KT * P, ap=[[KT * P, P], [1, KT * P]]
                ),
            )
            w_tiles[og] = wt

        # --- stages 1-3, chunked by batch chunk (NB rows). Stats stream x
        # in [P, XS] subchunks: bn_stats (mean/var) + max/min reduces ride
        # the same pass, so amax = max(max - mean, mean - min) needs no
        # second pass over x and c is ready before the signs.
        a_t = a_p.tile([P, KT, b_c], fp8)
        for h in range(BC):
            for bth in range(TPC):
                bt = h * TPC + bth
                st = small_p.tile([P, 2 * NQ, 6], f32, tag="bnst")
                mx4 = small_p.tile([P, NQ], f32, tag="mx4", name=f"mx{bth}")
                mn4 = small_p.tile([P, NQ], f32, tag="mn4", name=f"mn{bth}")
                for q in range(NQ):
                    if h == 0 and (bt, q) in x_pre:
                        xt_ = x_pre[(bt, q)]
                    else:
                        xt_ = xs_p.tile([P, XS], f16, tag="xs", name=f"xs{bt}_{q}")
                        nc.sync.dma_start(
                            out=xt_,
                            in_=x16[bt * P : (bt + 1) * P, q * XS : (q + 1) * XS],
                        )
                    xr = xt_.rearrange("p (n f) -> p n f", f=SC)
                    for i in range(XS // SC):
                        nc.vector.bn_stats(
                            out=st[:, q * (XS // SC) + i, :], in_=xr[:, i, :]
                        )
                    nc.vector.tensor_reduce(
                        out=mx4[:, q : q + 1], in_=xt_, axis=X, op=A.max
                    )
                    nc.vector.tensor_reduce(
                        out=mn4[:, q : q + 1], in_=xt_, axis=X, op=A.min
                    )
                mv = small_p.tile([P, 2], f32, tag="mv", name=f"mv{bth}")
                nc.vector.bn_aggr(out=mv, in_=st)
                mean = mv[:, 0:1]
                var = mv[:, 1:2]
                # fp16 mean for the sign path (signs tolerate fp16; see doc)
                m16 = small_p.tile([P, 1], f16, tag="m16")
                nc.scalar.activation(out=m16, in_=mean, func=AF.Copy)
                nc.sync.dma_start(out=mean_ds[h][bth * P : (bth + 1) * P], in_=m16)
                # c = max(max - mean, mean - min) * rsqrt(var + eps)
                mx = small_p.tile([P, 1], f32, tag="mx1")
                nc.vector.tensor_reduce(out=mx, in_=mx4, axis=X, op=A.max)
                mn = small_p.tile([P, 1], f32, tag="mn1")
                nc.vector.tensor_reduce(out=mn, in_=mn4, axis=X, op=A.min)
                t1 = small_p.tile([P, 1], f32, tag="t1")
                nc.vector.tensor_scalar(
                    out=t1, in0=mx, scalar1=mean, scalar2=None, op0=A.subtract
                )
                t2 = small_p.tile([P, 1], f32, tag="t2")
                nc.vector.tensor_sub(t2, mv[:, 0:1], mn)
                amax = small_p.tile([P, 1], f32, tag="amax")
                nc.vector.tensor_max(amax, t1, t2)
                std = small_p.tile([P, 1], f32, tag="std")
                nc.scalar.activation(out=std, in_=var, func=AF.Sqrt, bias=eps_t)
                rstd = small_p.tile([P, 1], f32, tag="rstd")
                nc.vector.reciprocal(rstd, std)
                cv = small_p.tile([P, 1], f32, tag="cv")
                nc.vector.tensor_mul(cv, amax, rstd)
                nc.sync.dma_start(out=c_ds[h][bth * P : (bth + 1) * P], in_=cv)

            # broadcast this chunk's mean + c across partitions
            mean_b = consts.tile([P, NB], f16, name=f"mean_b{h}")
            nc.sync.dma_start(
                out=mean_b,
                in_=bass.AP(tensor=mean_ds[h], offset=0, ap=[[0, P], [1, NB]]),
            )
            cb = consts.tile([P, NB], f32, name=f"cb{h}")
            nc.sync.dma_start(
                out=cb, in_=bass.AP(tensor=c_ds[h], offset=0, ap=[[0, P], [1, NB]])
            )
            if h == 0:
                cbs = [cb]
                mean_bs = [mean_b]
            else:
                cbs.append(cb)
                mean_bs.append(mean_b)

            # signs for this chunk, all k tiles (G k-tiles per DMA)
            for gi in range(KT // G):
                if h == 0 and gi in xtg_pre:
                    xtg = xtg_pre[gi]
                else:
                    xtg = xt_p.tile([P, G, NB], f16, tag="xtg")
                    nc.sync.dma_start(
                        out=xtg,
                        in_=bass.AP(
                            tensor=xTc,
                            offset=h * P * KT * NB + gi * G * NB,
                            ap=[[KT * NB, P], [1, G * NB]],
                        ),
                    )
                for r in range(G):
                    kt = gi * G + r
                    nc.vector.tensor_sub(xtg[:, r, :], xtg[:, r, :], mean_b)
                    nc.scalar.sign(
                        out=a_t[:, kt, h * NB : (h + 1) * NB], in_=xtg[:, r, :]
                    )

        # --- stage 4: matmul + epilogue, og-grouped so each group's weight
        # tiles stay resident across both batch chunks (weights stream from
        # HBM exactly once) and chunk-1 signs have a full group of slack.
        for grp in range(OG // GSZ):
            for bc in range(BC):
                for og in range(grp * GSZ, (grp + 1) * GSZ):
                    if og in w_tiles:
                        wt = w_tiles[og]
                    else:
                        wt = w_p.tile([P, KT * P], fp8, tag="w", name=f"w{og}")
                        nc.sync.dma_start(
                            out=wt,
                            in_=bass.AP(
                                tensor=wS,
                                offset=og * P * KT * P,
                                ap=[[KT * P, P], [1, KT * P]],
                            ),
                        )
                        w_tiles[og] = wt
                    w3 = wt.rearrange("p (kt oc) -> p kt oc", oc=P)
                    psum = ps_p.tile([P, NB], f32, tag=f"ps{bc}", name=f"psum{bc}")
                    for g in range(KT // 2):
                        nc.tensor.matmul(
                            psum,
                            lhsT=w3[:, 2 * g : 2 * g + 2, :],
                            rhs=a_t[:, 2 * g : 2 * g + 2, bc * NB : (bc + 1) * NB],
                            start=(g == 0),
                            stop=(g == KT // 2 - 1),
                            perf_mode=mybir.MatmulPerfMode.DoubleRow,
                        )
                    t1 = ep_p.tile([P, NB], f32, tag="t1")
                    nc.vector.tensor_tensor(
                        out=t1, in0=psum, in1=cbs[bc], op=A.mult
                    )
                    # out = t1*beta + bias*beta, per-partition scalars, on ACT
                    o16 = ep_p.tile([P, NB], f16, tag="o16")
                    nc.scalar.activation(
                        out=o16,
                        in_=t1,
                        func=AF.Identity,
                        bias=bb_t[:, og : og + 1],
                        scale=beta_t[:, og : og + 1],
                    )
                    nc.sync.dma_start(
                        out=outT[og * P : (og + 1) * P, bc * NB : (bc + 1) * NB],
                        in_=o16,
                    )
            if grp == 0:
                # drop group-0 weight tiles so the pool recycles buffers
                w_tiles.clear()

    return nc


def build_general_program(b_c, d_in, d_out):
    """bf16 fallback for gamma != 1: signs scaled by 1/gamma, no DoubleRow."""
    KT = d_in // P
    OG = d_out // P
    NB = 512
    BC = b_c // NB
    SC = min(512, d_in)
    nstat = d_in // SC

    f32 = mybir.dt.float32
    bf16 = mybir.dt.bfloat16
    X = mybir.AxisListType
    A = mybir.AluOpType
    AF = mybir.ActivationFunctionType
    G = min(4, KT)

    nc = bass.Bass("TRN2", target_bir_lowering=False, debug=False)
    x = nc.dram_tensor("x", [b_c, d_in], f32, kind="ExternalInput")
    xTc = nc.dram_tensor("xTc", [BC, P, KT, NB], f32, kind="ExternalInput")
    w4 = nc.dram_tensor("w4", [OG, P, KT, P], bf16, kind="ExternalInput")
    bias = nc.dram_tensor("bias", [d_out], f32, kind="ExternalInput")
    beta = nc.dram_tensor("beta", [d_out], f32, kind="ExternalInput")
    gamma = nc.dram_tensor("gamma", [d_in], f32, kind="ExternalInput")
    outT = nc.dram_tensor("outT", [d_out, b_c], f32, kind="ExternalOutput")
    mean_ds = [nc.dram_tensor(f"mean_d{h}", [NB], f32) for h in range(BC)]
    c_ds = [nc.dram_tensor(f"c_d{h}", [NB], f32) for h in range(BC)]

    XS = 1024
    NQ = d_in // XS

    with tile.TileContext(nc) as tc, ExitStack() as ctx:
        consts = ctx.enter_context(tc.tile_pool(name="consts", bufs=1))
        xs_p = ctx.enter_context(tc.tile_pool(name="xs", bufs=6))
        small_p = ctx.enter_context(tc.tile_pool(name="small", bufs=4))
        a_p = ctx.enter_context(tc.tile_pool(name="a", bufs=1))
        xt_p = ctx.enter_context(tc.tile_pool(name="xt", bufs=2))
        w_p = ctx.enter_context(tc.tile_pool(name="w", bufs=3))
        sw_p = ctx.enter_context(tc.tile_pool(name="sw", bufs=2))
        ep_p = ctx.enter_context(tc.tile_pool(name="ep", bufs=4))
        ps_p = ctx.enter_context(tc.tile_pool(name="ps", bufs=2 * BC, space="PSUM"))

        eps_t = consts.tile([P, 1], f32)
        nc.vector.memset(eps_t, EPS)
        bias_t = consts.tile([P, OG], f32)
        nc.sync.dma_start(
            out=bias_t, in_=bass.AP(tensor=bias, offset=0, ap=[[1, P], [P, OG]])
        )
        beta_t = consts.tile([P, OG], f32)
        nc.sync.dma_start(
            out=beta_t, in_=bass.AP(tensor=beta, offset=0, ap=[[1, P], [P, OG]])
        )
        bb_t = consts.tile([P, OG], f32)
        nc.vector.tensor_mul(bb_t, bias_t, beta_t)
        gamma_t = consts.tile([P, KT], f32)
        nc.sync.dma_start(
            out=gamma_t, in_=bass.AP(tensor=gamma, offset=0, ap=[[1, P], [P, KT]])
        )
        invg = consts.tile([P, KT], f32)
        nc.vector.reciprocal(invg, gamma_t)

        a_t = a_p.tile([P, KT, b_c], bf16)
        TPC = NB // P
        mean_bs = []
        cbs = []
        for h in range(BC):
            for bth in range(TPC):
                bt = h * TPC + bth
                st = small_p.tile([P, nstat, 6], f32, tag="bnst")
                mx4 = small_p.tile([P, NQ], f32, tag="mx4", name=f"mx{bth}")
                mn4 = small_p.tile([P, NQ], f32, tag="mn4", name=f"mn{bth}")
                for q in range(NQ):
                    xt_ = xs_p.tile([P, XS], f32, tag="xs", name=f"xs{bt}_{q}")
                    nc.sync.dma_start(
                        out=xt_,
                        in_=x[bt * P : (bt + 1) * P, q * XS : (q + 1) * XS],
                    )
                    xr = xt_.rearrange("p (n f) -> p n f", f=SC)
                    for i in range(XS // SC):
                        nc.vector.bn_stats(
                            out=st[:, q * (XS // SC) + i, :], in_=xr[:, i, :]
                        )
                    nc.vector.tensor_reduce(
                        out=mx4[:, q : q + 1], in_=xt_, axis=X.X, op=A.max
                    )
                    nc.vector.tensor_reduce(
                        out=mn4[:, q : q + 1], in_=xt_, axis=X.X, op=A.min
                    )
                mv = small_p.tile([P, 2], f32, tag="mv", name=f"mv{bth}")
                nc.vector.bn_aggr(out=mv, in_=st)
                nc.sync.dma_start(
                    out=mean_ds[h][bth * P : (bth + 1) * P], in_=mv[:, 0:1]
                )
                mx = small_p.tile([P, 1], f32, tag="mx1")
                nc.vector.tensor_reduce(out=mx, in_=mx4, axis=X.X, op=A.max)
                mn = small_p.tile([P, 1], f32, tag="mn1")
                nc.vector.tensor_reduce(out=mn, in_=mn4, axis=X.X, op=A.min)
                t1 = small_p.tile([P, 1], f32, tag="t1")
                nc.vector.tensor_scalar(
                    out=t1, in0=mx, scalar1=mv[:, 0:1], scalar2=None, op0=A.subtract
                )
                t2 = small_p.tile([P, 1], f32, tag="t2")
                nc.vector.tensor_sub(t2, mv[:, 0:1], mn)
                amax = small_p.tile([P, 1], f32, tag="amax")
                nc.vector.tensor_max(amax, t1, t2)
                std = small_p.tile([P, 1], f32, tag="std")
                nc.scalar.activation(out=std, in_=mv[:, 1:2], func=AF.Sqrt, bias=eps_t)
                rstd = small_p.tile([P, 1], f32, tag="rstd")
                nc.vector.reciprocal(rstd, std)
                cv = small_p.tile([P, 1], f32, tag="cv")
                nc.vector.tensor_mul(cv, amax, rstd)
                nc.sync.dma_start(out=c_ds[h][bth * P : (bth + 1) * P], in_=cv)

            mean_b = consts.tile([P, NB], f32, name=f"mean_b{h}")
            nc.sync.dma_start(
                out=mean_b,
                in_=bass.AP(tensor=mean_ds[h], offset=0, ap=[[0, P], [1, NB]]),
            )
            mean_bs.append(mean_b)
            cb = consts.tile([P, NB], f32, name=f"cb{h}")
            nc.sync.dma_start(
                out=cb, in_=bass.AP(tensor=c_ds[h], offset=0, ap=[[0, P], [1, NB]])
            )
            cbs.append(cb)

            for gi in range(KT // G):
                xtg = xt_p.tile([P, G, NB], f32, tag="xtg")
                nc.sync.dma_start(
                    out=xtg,
                    in_=bass.AP(
                        tensor=xTc,
                        offset=h * P * KT * NB + gi * G * NB,
                        ap=[[KT * NB, P], [1, G * NB]],
                    ),
                )
                for r in range(G):
                    kt = gi * G + r
                    nc.vector.tensor_sub(xtg[:, r, :], xtg[:, r, :], mean_b)
                    stmp = xt_p.tile([P, NB], bf16, tag="stmp")
                    nc.scalar.sign(out=stmp, in_=xtg[:, r, :])
                    nc.vector.tensor_scalar_mul(
                        out=a_t[:, kt, h * NB : (h + 1) * NB],
                        in0=stmp,
                        scalar1=invg[:, kt : kt + 1],
                    )

        for og in range(OG):
            wcol = w_p.tile([P, KT * P], bf16, tag="wcol")
            nc.sync.dma_start(
                out=wcol,
                in_=bass.AP(
                    tensor=w4, offset=og * P * KT * P, ap=[[KT * P, P], [1, KT * P]]
                ),
            )
            wcol3 = wcol.rearrange("p (kt oc) -> p kt oc", oc=P)
            sw = sw_p.tile([P, KT, P], bf16, tag="sw")
            nc.scalar.sign(out=sw, in_=wcol3)
            psums = [
                ps_p.tile([P, NB], f32, tag=f"ps{bc}", name=f"psum{bc}")
                for bc in range(BC)
            ]
            for bc in range(BC):
                for kt in range(KT):
                    nc.tensor.matmul(
                        psums[bc],
                        lhsT=sw[:, kt, :],
                        rhs=a_t[:, kt, bc * NB : (bc + 1) * NB],
                        start=(kt == 0),
                        stop=(kt == KT - 1),
                    )
            for bc in range(BC):
                t1 = ep_p.tile([P, NB], f32, tag="t1")
                nc.vector.tensor_tensor(out=t1, in0=psums[bc], in1=cbs[bc], op=A.mult)
                o_sb = ep_p.tile([P, NB], f32, tag="osb")
                nc.scalar.activation(
                    out=o_sb,
                    in_=t1,
                    func=AF.Identity,
                    bias=bb_t[:, og : og + 1],
                    scale=beta_t[:, og : og + 1],
                )
                nc.sync.dma_start(
                    out=outT[og * P : (og + 1) * P, bc * NB : (bc + 1) * NB],
                    in_=o_sb,
                )

    return nc


def kernel(input, weight, bias, gamma, beta, _run_kwargs=None):
    import ml_dtypes

    input = np.ascontiguousarray(np.asarray(input, dtype=np.float32))
    weight = np.ascontiguousarray(np.asarray(weight, dtype=np.float32))
    bias = np.ascontiguousarray(np.asarray(bias, dtype=np.float32))
    gamma = np.ascontiguousarray(np.asarray(gamma, dtype=np.float32))
    beta = np.ascontiguousarray(np.asarray(beta, dtype=np.float32))

    B, d_in = input.shape
    d_out = weight.shape[0]
    assert B % N_CORES == 0
    b_c = B // N_CORES
    OG, KT = d_out // 128, d_in // 128
    NB = 512
    BC = b_c // NB

    fast = bool(np.all(gamma == 1.0))

    if fast:
        nc = build_fast_program(b_c, d_in, d_out)
        fp8np = mybir.dt.np(mybir.dt.float8e4)
        # wS[og, p, kt, oc] = sign(W[og*128+oc, kt*128+p]), exact in fp8e4
        wS = np.ascontiguousarray(
            np.sign(weight).reshape(OG, 128, KT, 128).transpose(0, 3, 2, 1)
        ).astype(fp8np)
        x16_full = input.astype(np.float16)
        in_maps = []
        for c in range(N_CORES):
            x_c = x16_full[c * b_c : (c + 1) * b_c, :]
            xTc = np.ascontiguousarray(
                x_c.reshape(BC, NB, KT, 128).transpose(0, 3, 2, 1)
            )
            in_maps.append(
                {
                    "x16": np.ascontiguousarray(x_c),
                    "xTc": xTc,
                    "wS": wS,
                    "bias": bias,
                    "beta": beta,
                }
            )
    else:
        nc = build_general_program(b_c, d_in, d_out)
        w4 = np.ascontiguousarray(
            weight.reshape(OG, 128, KT, 128).transpose(0, 3, 2, 1)
        ).astype(ml_dtypes.bfloat16)
        in_maps = []
        for c in range(N_CORES):
            x_c = np.ascontiguousarray(input[c * b_c : (c + 1) * b_c, :])
            xTc = np.ascontiguousarray(
                x_c.reshape(BC, NB, KT, 128).transpose(0, 3, 2, 1)
            )
            in_maps.append(
                {
                    "x": x_c,
                    "xTc": xTc,
                    "w4": w4,
                    "bias": bias,
                    "beta": beta,
                    "gamma": gamma,
                }
            )

    res = run_bass_kernel_spmd(
        nc, in_maps, core_ids=list(range(N_CORES)), **(_run_kwargs or {})
    )

    out = np.empty((B, d_out), dtype=np.float32)
    for c in range(N_CORES):
        out[c * b_c : (c + 1) * b_c, :] = res.results[c]["outT"].T.astype(np.float32)
    if _run_kwargs:
        kernel.last_results = res
    return out


# revision 8
# speedup vs baseline: 1.1795x; 1.1795x over previous
"""BitLinear (layernorm -> absmax sign-quant -> sign-weight matmul -> bias*beta)
for Trainium2, batch-sharded across 8 NeuronCores.

Math (per row b, feature i, output o):
    mean_b  = mean(x[b,:]);  var_b = var(x[b,:])
    c_b     = max_i |x[b,i] - mean_b| * rsqrt(var_b + eps)
    A[b,i]  = sign(x[b,i] - mean_b)           (sign(xn) == sign(x - mean))
    out[b,o]= (c_b * sum_i A[b,i]*sign(W[o,i]) + bias[o]) * beta[o]

Fast path (gamma == 1): weight signs are computed on the host and shipped as
fp8e4 (+-1 exact), x is shipped as fp16 in both natural and transposed
layouts (sign flips from fp16 rounding cost ~6e-3 rel err, under the 2e-2
gate), the big GEMM runs fp8 DoubleRow (2 MACs/cell/cycle), absmax comes from
streaming max/min of raw x (amax = max(max-mean, mean-min)), and the output
is stored fp16 and upcast on the host. Each core handles 1024 batch rows; no
collectives. The general path (gamma != 1) keeps the slower bf16 pipeline.
"""
import sys

sys.path.insert(0, "/opt/trn_rl_repo")

from contextlib import ExitStack

import numpy as np

import concourse.bass as bass
import concourse.tile as tile
from concourse import mybir
from concourse.bass_utils import run_bass_kernel_spmd
from concourse.vector_clock import ScopedClock, VectorClock

N_CORES = 8
EPS = 1e-5
P = 128


# ---------------------------------------------------------------------------
# Workaround: this walrus build rejects CTRL instructions (Drain/NoOp) with
# more than one sync wait. Tile's final drain carries one wait per live
# processor. Split them across single-wait SP nops; SP program order makes
# this equivalent.
def _patched_drain_and_barrier(self, tick_clock, wait_clock):
    gc = tick_clock.global_clock
    for scope, vclock in ScopedClock({None: gc}).items():
        n = len(vclock)
        for i in range(n):
            if vclock[i] > 0:
                vec = [0] * n
                vec[i] = vclock[i]
                nop_inst = self.nc.sync.nop(nofuse=True, hint="split_drain_wait")
                wait_clock.add_sem_waits(
                    nop_inst.ins, ScopedClock({scope: VectorClock(vec)})
                )
    self.nc.sync.drain()
    self.nc.all_engine_barrier()
    assert self.sems is not None
    popped = self.nc._tile_sem_poison_stack.pop()
    assert popped is self._sem_poison
    self.nc.clear_and_free_semaphores(list(self.sems.allocated().values()))
    self.nc.all_engine_barrier()


tile.TileContext._drain_and_barrier = _patched_drain_and_barrier


# This walrus build allows at most ONE sync wait on ANY instruction. Tile's
# wait-assignment emits up to 4. Post-process the serialized BIR: move all but
# the last wait of each instruction onto same-engine NoOps placed just before
# it (engine program order preserves semantics; for DMAs this gates descriptor
# submission, which is strictly more conservative).
def _split_multi_waits(m: dict) -> dict:
    for fn in m["functions"]:
        for bb in fn["blocks"]:
            out = []
            for ins in bb["instructions"]:
                si = ins.get("sync_info") or {}
                waits = si.get("on_wait") or []
                if len(waits) > 1:
                    for i, w in enumerate(waits[:-1]):
                        out.append(
                            {
                                "debug": ins.get("debug", 0),
                                "engine": ins["engine"],
                                "ins": [],
                                "outs": [],
                                "name": f"{ins['name']}-w{i}",
                                "opcode": "NoOp",
                                "sync_info": {"on_update": [], "on_wait": [w]},
                                "text_hint": "split_wait",
                            }
                        )
                    si["on_wait"] = [waits[-1]]
                out.append(ins)
            bb["instructions"] = out
    return m


_orig_to_json_bytes = bass.Bass.to_json_bytes


def _patched_to_json_bytes(self):
    import orjson

    m = orjson.loads(_orig_to_json_bytes(self))
    return orjson.dumps(_split_multi_waits(m))


bass.Bass.to_json_bytes = _patched_to_json_bytes
# ---------------------------------------------------------------------------


def build_fast_program(b_c, d_in, d_out):
    """fp8 DoubleRow fast path (gamma == 1, any beta/bias). fp16 x, fp16 out.

    DMA discipline: every dma_start dispatches serially through the sync
    engine (~0.6us each, 8 outstanding slots) and each descriptor costs
    ~155ns regardless of size, so transfers are merged into few instructions
    with >=4KB per-partition runs: 2 half-loads per x batch-tile, G=8 k-tiles
    per transposed load, 2 weight columns per load, and one batched
    store+broadcast per chunk for the mean/c roundtrips.
    """
    KT = d_in // P  # contraction tiles (32)
    OG = d_out // P  # output-feature tiles (32)
    NB = 512  # matmul moving free dim = one PSUM bank of fp32
    BC = b_c // NB  # batch chunks (2)
    TPC = NB // P  # batch tiles per chunk (4)
    SC = 512  # bn_stats hardware max free size
    G = 8  # k-tiles per transposed-input DMA
    GSZ = 16  # ogs per matmul group (weight pairs resident per group)

    f32 = mybir.dt.float32
    f16 = mybir.dt.float16
    fp8 = mybir.dt.float8e4
    X = mybir.AxisListType.X
    A = mybir.AluOpType
    AF = mybir.ActivationFunctionType

    nc = bass.Bass("TRN2", target_bir_lowering=False, debug=False)
    x16 = nc.dram_tensor("x16", [b_c, d_in], f16, kind="ExternalInput")
    # host-prechunked transpose: xTc[h, p, kt, j] = x[h*NB + j, kt*128 + p]
    xTc = nc.dram_tensor("xTc", [BC, P, KT, NB], f16, kind="ExternalInput")
    # host-pretiled weight signs: wS[og, p, kt, oc] = sign(W[og*128+oc, kt*128+p])
    wS = nc.dram_tensor("wS", [OG, P, KT, P], fp8, kind="ExternalInput")
    bias = nc.dram_tensor("bias", [d_out], f32, kind="ExternalInput")
    beta = nc.dram_tensor("beta", [d_out], f32, kind="ExternalInput")
    outT = nc.dram_tensor("outT", [d_out, b_c], f16, kind="ExternalOutput")
    mean_ds = [nc.dram_tensor(f"mean_d{h}", [NB], f16) for h in range(BC)]
    c_ds = [nc.dram_tensor(f"c_d{h}", [NB], f32) for h in range(BC)]

    with tile.TileContext(nc) as tc, ExitStack() as ctx:
        consts = ctx.enter_context(tc.tile_pool(name="consts", bufs=1))
        xs_p = ctx.enter_context(tc.tile_pool(name="xs", bufs=6))
        small_p = ctx.enter_context(tc.tile_pool(name="small", bufs=4))
        a_p = ctx.enter_context(tc.tile_pool(name="a", bufs=1))
        xt_p = ctx.enter_context(tc.tile_pool(name="xt", bufs=3))
        w_p = ctx.enter_context(tc.tile_pool(name="w", bufs=10))
        ep_p = ctx.enter_context(tc.tile_pool(name="ep", bufs=4))
        ps_p = ctx.enter_context(tc.tile_pool(name="ps", bufs=6, space="PSUM"))

        # --- constants ---------------------------------------------------
        eps_t = consts.tile([P, 1], f32)
        nc.vector.memset(eps_t, EPS)
        bias_t = consts.tile([P, OG], f32)
        nc.sync.dma_start(
            out=bias_t, in_=bass.AP(tensor=bias, offset=0, ap=[[1, P], [P, OG]])
        )
        beta_t = consts.tile([P, OG], f32)
        nc.sync.dma_start(
            out=beta_t, in_=bass.AP(tensor=beta, offset=0, ap=[[1, P], [P, OG]])
        )
        bb_t = consts.tile([P, OG], f32)
        nc.vector.tensor_mul(bb_t, bias_t, beta_t)

        a_t = a_p.tile([P, KT, b_c], fp8)
        HS = d_in // 2

        def emit_x_loads(h):
            tiles = []
            for bth in range(TPC):
                bt = h * TPC + bth
                xt_ = xs_p.tile([P, d_in], f16, tag="xs", name=f"xs{bt}")
                for q in range(2):
                    nc.sync.dma_start(
                        out=xt_[:, q * HS : (q + 1) * HS],
                        in_=x16[bt * P : (bt + 1) * P, q * HS : (q + 1) * HS],
                    )
                tiles.append(xt_)
            return tiles

        def emit_xtg_loads(h, gis):
            out = {}
            for gi in gis:
                xtg = xt_p.tile([P, G, NB], f16, tag="xtg", name=f"xtg{h}_{gi}")
                nc.sync.dma_start(
                    out=xtg,
                    in_=bass.AP(
                        tensor=xTc,
                        offset=h * P * KT * NB + gi * G * NB,
                        ap=[[KT * NB, P], [1, G * NB]],
                    ),
                )
                out[gi] = xtg
            return out

        def emit_w_pair(pr):
            wt = w_p.tile([P, 2, KT, P], fp8, tag="w", name=f"wp{pr}")
            nc.sync.dma_start(
                out=wt,
                in_=bass.AP(
                    tensor=wS,
                    offset=pr * 2 * P * KT * P,
                    ap=[[KT * P, P], [P * KT * P, 2], [1, KT * P]],
                ),
            )
            return wt

        # --- prologue: chunk-0 inputs + first transposed groups + first
        # weight pairs own the DMA queue heads.
        x0_tiles = emit_x_loads(0)
        xtg0_pre = emit_xtg_loads(0, range(2))
        w_tiles = {pr: emit_w_pair(pr) for pr in range(6)}

        def emit_stats(h, x_tiles):
            """bn mean/var per btile; pack fp16 means into m4 (one column per
            btile) for a single batched store."""
            m4 = consts.tile([P, TPC], f16, name=f"m4_{h}")
            mvs = []
            for bth in range(TPC):
                xt_ = x_tiles[bth]
                xr = xt_.rearrange("p (n f) -> p n f", f=SC)
                st = small_p.tile([P, d_in // SC, 6], f32, tag="bnst")
                for i in range(d_in // SC):
                    nc.vector.bn_stats(out=st[:, i, :], in_=xr[:, i, :])
                mv = small_p.tile([P, 2], f32, tag="mv", name=f"mv{h}_{bth}")
                nc.vector.bn_aggr(out=mv, in_=st)
                nc.scalar.activation(
                    out=m4[:, bth : bth + 1], in_=mv[:, 0:1], func=AF.Copy
                )
                mvs.append(mv)
            return m4, mvs

        def emit_mean_bcast(h, m4):
            nc.sync.dma_start(
                out=bass.AP(tensor=mean_ds[h], offset=0, ap=[[1, P], [P, TPC]]),
                in_=m4,
            )
            mean_b = consts.tile([P, NB], f16, name=f"mean_b{h}")
            nc.sync.dma_start(
                out=mean_b,
                in_=bass.AP(tensor=mean_ds[h], offset=0, ap=[[0, P], [1, NB]]),
            )
            return mean_b

        def emit_signs(h, mean_b, xtg_pre):
            for gi in range(KT // G):
                if gi in xtg_pre:
                    xtg = xtg_pre[gi]
                else:
                    xtg = emit_xtg_loads(h, [gi])[gi]
                for r in range(G):
                    kt = gi * G + r
                    nc.vector.tensor_sub(xtg[:, r, :], xtg[:, r, :], mean_b)
                    nc.scalar.sign(
                        out=a_t[:, kt, h * NB : (h + 1) * NB], in_=xtg[:, r, :]
                    )

        def emit_cscale(h, x_tiles, mvs):
            """c = max(max-mean, mean-min) * rsqrt(var+eps); single-op max and
            min reduces over the full row, deferred off the sign critical
            path; batched store + broadcast."""
            c4 = consts.tile([P, TPC], f32, name=f"c4_{h}")
            for bth in range(TPC):
                xt_ = x_tiles[bth]
                mv = mvs[bth]
                mx = small_p.tile([P, 1], f32, tag="mx1")
                nc.vector.tensor_reduce(out=mx, in_=xt_, axis=X, op=A.max)
                mn = small_p.tile([P, 1], f32, tag="mn1")
                nc.vector.tensor_reduce(out=mn, in_=xt_, axis=X, op=A.min)
                t1 = small_p.tile([P, 1], f32, tag="t1")
                nc.vector.tensor_scalar(
                    out=t1, in0=mx, scalar1=mv[:, 0:1], scalar2=None, op0=A.subtract
                )
                t2 = small_p.tile([P, 1], f32, tag="t2")
                nc.vector.tensor_sub(t2, mv[:, 0:1], mn)
                amax = small_p.tile([P, 1], f32, tag="amax")
                nc.vector.tensor_max(amax, t1, t2)
                std = small_p.tile([P, 1], f32, tag="std")
                nc.scalar.activation(
                    out=std, in_=mv[:, 1:2], func=AF.Sqrt, bias=eps_t
                )
                rstd = small_p.tile([P, 1], f32, tag="rstd")
                nc.vector.reciprocal(rstd, std)
                nc.vector.tensor_mul(c4[:, bth : bth + 1], amax, rstd)
            nc.sync.dma_start(
                out=bass.AP(tensor=c_ds[h], offset=0, ap=[[1, P], [P, TPC]]),
                in_=c4,
            )
            cb = consts.tile([P, NB], f32, name=f"cb{h}")
            nc.sync.dma_start(
                out=cb, in_=bass.AP(tensor=c_ds[h], offset=0, ap=[[0, P], [1, NB]])
            )
            return cb

        # --- chunk 0: stats -> mean bcast -> signs; then prefetch chunk-1
        # inputs BEFORE the c-scale block so their dispatch is not gated on
        # the c0 reduce chain (the sync queue is FIFO).
        m4_0, mvs0 = emit_stats(0, x0_tiles)
        mean_b0 = emit_mean_bcast(0, m4_0)
        emit_signs(0, mean_b0, xtg0_pre)
        x1_tiles = emit_x_loads(1)
        xtg1_pre = emit_xtg_loads(1, range(2))
        cb0 = emit_cscale(0, x0_tiles, mvs0)
        m4_1, mvs1 = emit_stats(1, x1_tiles)
        mean_b1 = emit_mean_bcast(1, m4_1)
        emit_signs(1, mean_b1, xtg1_pre)
        cb1 = emit_cscale(1, x1_tiles, mvs1)
        cbs = [cb0, cb1]

        # --- matmul + epilogue: og-grouped so each group's weight pairs stay
        # resident across both batch chunks (weights stream from HBM once)
        # and chunk-1 signs have a full group of slack.
        for grp in range(OG // GSZ):
            for bc in range(BC):
                for og in range(grp * GSZ, (grp + 1) * GSZ):
                    pr, half = og // 2, og % 2
                    if pr not in w_tiles:
                        w_tiles[pr] = emit_w_pair(pr)
                    wt = w_tiles[pr]
                    psum = ps_p.tile([P, NB], f32, tag="ps", name=f"ps{og}_{bc}")
                    for g in range(KT // 2):
                        nc.tensor.matmul(
                            psum,
                            lhsT=wt[:, half, 2 * g : 2 * g + 2, :],
                            rhs=a_t[:, 2 * g : 2 * g + 2, bc * NB : (bc + 1) * NB],
                            start=(g == 0),
                            stop=(g == KT // 2 - 1),
                            perf_mode=mybir.MatmulPerfMode.DoubleRow,
                        )
                    t1 = ep_p.tile([P, NB], f32, tag="t1")
                    nc.vector.tensor_tensor(
                        out=t1, in0=psum, in1=cbs[bc], op=A.mult
                    )
                    o16 = ep_p.tile([P, NB], f16, tag="o16")
                    nc.scalar.activation(
                        out=o16,
                        in_=t1,
                        func=AF.Identity,
                        bias=bb_t[:, og : og + 1],
                        scale=beta_t[:, og : og + 1],
                    )
                    nc.sync.dma_start(
                        out=outT[og * P : (og + 1) * P, bc * NB : (bc + 1) * NB],
                        in_=o16,
                    )
            if grp == 0:
                w_tiles.clear()

    return nc


def build_general_program(b_c, d_in, d_out):
    """bf16 fallback for gamma != 1: signs scaled by 1/gamma, no DoubleRow."""
    KT = d_in // P
    OG = d_out // P
    NB = 512
    BC = b_c // NB
    SC = min(512, d_in)
    nstat = d_in // SC

    f32 = mybir.dt.float32
    bf16 = mybir.dt.bfloat16
    X = mybir.AxisListType
    A = mybir.AluOpType
    AF = mybir.ActivationFunctionType
    G = min(4, KT)

    nc = bass.Bass("TRN2", target_bir_lowering=False, debug=False)
    x = nc.dram_tensor("x", [b_c, d_in], f32, kind="ExternalInput")
    xTc = nc.dram_tensor("xTc", [BC, P, KT, NB], f32, kind="ExternalInput")
    w4 = nc.dram_tensor("w4", [OG, P, KT, P], bf16, kind="ExternalInput")
    bias = nc.dram_tensor("bias", [d_out], f32, kind="ExternalInput")
    beta = nc.dram_tensor("beta", [d_out], f32, kind="ExternalInput")
    gamma = nc.dram_tensor("gamma", [d_in], f32, kind="ExternalInput")
    outT = nc.dram_tensor("outT", [d_out, b_c], f32, kind="ExternalOutput")
    mean_ds = [nc.dram_tensor(f"mean_d{h}", [NB], f32) for h in range(BC)]
    c_ds = [nc.dram_tensor(f"c_d{h}", [NB], f32) for h in range(BC)]

    XS = 1024
    NQ = d_in // XS

    with tile.TileContext(nc) as tc, ExitStack() as ctx:
        consts = ctx.enter_context(tc.tile_pool(name="consts", bufs=1))
        xs_p = ctx.enter_context(tc.tile_pool(name="xs", bufs=6))
        small_p = ctx.enter_context(tc.tile_pool(name="small", bufs=4))
        a_p = ctx.enter_context(tc.tile_pool(name="a", bufs=1))
        xt_p = ctx.enter_context(tc.tile_pool(name="xt", bufs=2))
        w_p = ctx.enter_context(tc.tile_pool(name="w", bufs=3))
        sw_p = ctx.enter_context(tc.tile_pool(name="sw", bufs=2))
        ep_p = ctx.enter_context(tc.tile_pool(name="ep", bufs=4))
        ps_p = ctx.enter_context(tc.tile_pool(name="ps", bufs=2 * BC, space="PSUM"))

        eps_t = consts.tile([P, 1], f32)
        nc.vector.memset(eps_t, EPS)
        bias_t = consts.tile([P, OG], f32)
        nc.sync.dma_start(
            out=bias_t, in_=bass.AP(tensor=bias, offset=0, ap=[[1, P], [P, OG]])
        )
        beta_t = consts.tile([P, OG], f32)
        nc.sync.dma_start(
            out=beta_t, in_=bass.AP(tensor=beta, offset=0, ap=[[1, P], [P, OG]])
        )
        bb_t = consts.tile([P, OG], f32)
        nc.vector.tensor_mul(bb_t, bias_t, beta_t)
        gamma_t = consts.tile([P, KT], f32)
        nc.sync.dma_start(
            out=gamma_t, in_=bass.AP(tensor=gamma, offset=0, ap=[[1, P], [P, KT]])
        )
        invg = consts.tile([P, KT], f32)
        nc.vector.reciprocal(invg, gamma_t)

        a_t = a_p.tile([P, KT, b_c], bf16)
        TPC = NB // P
        mean_bs = []
        cbs = []
        for h in range(BC):
            for bth in range(TPC):
                bt = h * TPC + bth
                st = small_p.tile([P, nstat, 6], f32, tag="bnst")
                mx4 = small_p.tile([P, NQ], f32, tag="mx4", name=f"mx{bth}")
                mn4 = small_p.tile([P, NQ], f32, tag="mn4", name=f"mn{bth}")
                for q in range(NQ):
                    xt_ = xs_p.tile([P, XS], f32, tag="xs", name=f"xs{bt}_{q}")
                    nc.sync.dma_start(
                        out=xt_,
                        in_=x[bt * P : (bt + 1) * P, q * XS : (q + 1) * XS],
                    )
                    xr = xt_.rearrange("p (n f) -> p n f", f=SC)
                    for i in range(XS // SC):
                        nc.vector.bn_stats(
                            out=st[:, q * (XS // SC) + i, :], in_=xr[:, i, :]
                        )
                    nc.vector.tensor_reduce(
                        out=mx4[:, q : q + 1], in_=xt_, axis=X.X, op=A.max
                    )
                    nc.vector.tensor_reduce(
                        out=mn4[:, q : q + 1], in_=xt_, axis=X.X, op=A.min
                    )
                mv = small_p.tile([P, 2], f32, tag="mv", name=f"mv{bth}")
                nc.vector.bn_aggr(out=mv, in_=st)
                nc.sync.dma_start(
                    out=mean_ds[h][bth * P : (bth + 1) * P], in_=mv[:, 0:1]
                )
                mx = small_p.tile([P, 1], f32, tag="mx1")
                nc.vector.tensor_reduce(out=mx, in_=mx4, axis=X.X, op=A.max)
                mn = small_p.tile([P, 1], f32, tag="mn1")
                nc.vector.tensor_reduce(out=mn, in_=mn4, axis=X.X, op=A.min)
                t1 = small_p.tile([P, 1], f32, tag="t1")
                nc.vector.tensor_scalar(
                    out=t1, in0=mx, scalar1=mv[:, 0:1], scalar2=None, op0=A.subtract
                )
                t2 = small_p.tile([P, 1], f32, tag="t2")
                nc.vector.tensor_sub(t2, mv[:, 0:1], mn)
                amax = small_p.tile([P, 1], f32, tag="amax")
                nc.vector.tensor_max(amax, t1, t2)
                std = small_p.tile([P, 1], f32, tag="std")
                nc.scalar.activation(out=std, in_=mv[:, 1:2], func=AF.Sqrt, bias=eps_t)
                rstd = small_p.tile([P, 1], f32, tag="rstd")
                nc.vector.reciprocal(rstd, std)
                cv = small_p.tile([P, 1], f32, tag="cv")
                nc.vector.tensor_mul(cv, amax, rstd)
                nc.sync.dma_start(out=c_ds[h][bth * P : (bth + 1) * P], in_=cv)

            mean_b = consts.tile([P, NB], f32, name=f"mean_b{h}")
            nc.sync.dma_start(
                out=mean_b,
                in_=bass.AP(tensor=mean_ds[h], offset=0, ap=[[0, P], [1, NB]]),
            )
            mean_bs.append(mean_b)
            cb = consts.tile([P, NB], f32, name=f"cb{h}")
            nc.sync.dma_start(
                out=cb, in_=bass.AP(tensor=c_ds[h], offset=0, ap=[[0, P], [1, NB]])
            )
            cbs.append(cb)

            for gi in range(KT // G):
                xtg = xt_p.tile([P, G, NB], f32, tag="xtg")
                nc.sync.dma_start(
                    out=xtg,
                    in_=bass.AP(
                        tensor=xTc,
                        offset=h * P * KT * NB + gi * G * NB,
                        ap=[[KT * NB, P], [1, G * NB]],
                    ),
                )
                for r in range(G):
                    kt = gi * G + r
                    nc.vector.tensor_sub(xtg[:, r, :], xtg[:, r, :], mean_b)
                    stmp = xt_p.tile([P, NB], bf16, tag="stmp")
                    nc.scalar.sign(out=stmp, in_=xtg[:, r, :])
                    nc.vector.tensor_scalar_mul(
                        out=a_t[:, kt, h * NB : (h + 1) * NB],
                        in0=stmp,
                        scalar1=invg[:, kt : kt + 1],
                    )

        for og in range(OG):
            wcol = w_p.tile([P, KT * P], bf16, tag="wcol")
            nc.sync.dma_start(
                out=wcol,
                in_=bass.AP(
                    tensor=w4, offset=og * P * KT * P, ap=[[KT * P, P], [1, KT * P]]
                ),
            )
            wcol3 = wcol.rearrange("p (kt oc) -> p kt oc", oc=P)
            sw = sw_p.tile([P, KT, P], bf16, tag="sw")
            nc.scalar.sign(out=sw, in_=wcol3)
            psums = [
                ps_p.tile([P, NB], f32, tag=f"ps{bc}", name=f"psum{bc}")
                for bc in range(BC)
            ]
            for bc in range(BC):
                for kt in range(KT):
                    nc.tensor.matmul(
                        psums[bc],
                        lhsT=sw[:, kt, :],
                        rhs=a_t[:, kt, bc * NB : (bc + 1) * NB],
                        start=(kt == 0),
                        stop=(kt == KT - 1),
                    )
            for bc in range(BC):
                t1 = ep_p.tile([P, NB], f32, tag="t1")
                nc.vector.tensor_tensor(out=t1, in0=psums[bc], in1=cbs[bc], op=A.mult)
                o_sb = ep_p.tile([P, NB], f32, tag="osb")
                nc.scalar.activation(
                    out=o_sb,
                    in_=t1,
                    func=AF.Identity,
                    bias=bb_t[:, og : og + 1],
                    scale=beta_t[:, og : og + 1],
                )
                nc.sync.dma_start(
                    out=outT[og * P : (og + 1) * P, bc * NB : (bc + 1) * NB],
                    in_=o_sb,
                )

    return nc


def kernel(input, weight, bias, gamma, beta, _run_kwargs=None):
    import ml_dtypes

    input = np.ascontiguousarray(np.asarray(input, dtype=np.float32))
    weight = np.ascontiguousarray(np.asarray(weight, dtype=np.float32))
    bias = np.ascontiguousarray(np.asarray(bias, dtype=np.float32))
    gamma = np.ascontiguousarray(np.asarray(gamma, dtype=np.float32))
    beta = np.ascontiguousarray(np.asarray(beta, dtype=np.float32))

    B, d_in = input.shape
    d_out = weight.shape[0]
    assert B % N_CORES == 0
    b_c = B // N_CORES
    OG, KT = d_out // 128, d_in // 128
    NB = 512
    BC = b_c // NB

    fast = bool(np.all(gamma == 1.0))

    if fast:
        nc = build_fast_program(b_c, d_in, d_out)
        fp8np = mybir.dt.np(mybir.dt.float8e4)
        # wS[og, p, kt, oc] = sign(W[og*128+oc, kt*128+p]), exact in fp8e4
        wS = np.ascontiguousarray(
            np.sign(weight).reshape(OG, 128, KT, 128).transpose(0, 3, 2, 1)
        ).astype(fp8np)
        x16_full = input.astype(np.float16)
        in_maps = []
        for c in range(N_CORES):
            x_c = x16_full[c * b_c : (c + 1) * b_c, :]
            xTc = np.ascontiguousarray(
                x_c.reshape(BC, NB, KT, 128).transpose(0, 3, 2, 1)
            )
            in_maps.append(
                {
                    "x16": np.ascontiguousarray(x_c),
                    "xTc": xTc,
                    "wS": wS,
                    "bias": bias,
                    "beta": beta,
                }
            )
    else:
        nc = build_general_program(b_c, d_in, d_out)
        w4 = np.ascontiguousarray(
            weight.reshape(OG, 128, KT, 128).transpose(0, 3, 2, 1)
        ).astype(ml_dtypes.bfloat16)
        in_maps = []
        for c in range(N_CORES):
            x_c = np.ascontiguousarray(input[c * b_c : (c + 1) * b_c, :])
            xTc = np.ascontiguousarray(
                x_c.reshape(BC, NB, KT, 128).transpose(0, 3, 2, 1)
            )
            in_maps.append(
                {
                    "x": x_c,
                    "xTc": xTc,
                    "w4": w4,
                    "bias": bias,
                    "beta": beta,
                    "gamma": gamma,
                }
            )

    res = run_bass_kernel_spmd(
        nc, in_maps, core_ids=list(range(N_CORES)), **(_run_kwargs or {})
    )

    out = np.empty((B, d_out), dtype=np.float32)
    for c in range(N_CORES):
        out[c * b_c : (c + 1) * b_c, :] = res.results[c]["outT"].T.astype(np.float32)
    if _run_kwargs:
        kernel.last_results = res
    return out


# revision 10
# speedup vs baseline: 1.3420x; 1.1377x over previous
"""BitLinear (layernorm -> absmax sign-quant -> sign-weight matmul -> bias*beta)
for Trainium2, batch-sharded across 8 NeuronCores.

Math (per row b, feature i, output o):
    mean_b  = mean(x[b,:]);  var_b = var(x[b,:])
    c_b     = max_i |x[b,i] - mean_b| * rsqrt(var_b + eps)
    A[b,i]  = sign(x[b,i] - mean_b)           (sign(xn) == sign(x - mean))
    out[b,o]= (c_b * sum_i A[b,i]*sign(W[o,i]) + bias[o]) * beta[o]

Fast path (gamma == 1): weight signs are computed on the host and shipped as
fp8e4 (+-1 exact), x is shipped as fp16 in both natural and transposed
layouts (sign flips from fp16 rounding cost ~6e-3 rel err, under the 2e-2
gate), the big GEMM runs fp8 DoubleRow (2 MACs/cell/cycle), absmax comes from
streaming max/min of raw x (amax = max(max-mean, mean-min)), and the output
is stored fp16 and upcast on the host. Each core handles 1024 batch rows; no
collectives. The general path (gamma != 1) keeps the slower bf16 pipeline.
"""
import sys

sys.path.insert(0, "/opt/trn_rl_repo")

from contextlib import ExitStack

import numpy as np

import concourse.bass as bass
import concourse.tile as tile
from concourse import masks, mybir
from concourse.bass_utils import run_bass_kernel_spmd
from concourse.vector_clock import ScopedClock, VectorClock

N_CORES = 8
EPS = 1e-5
P = 128


# ---------------------------------------------------------------------------
# Workaround: this walrus build rejects CTRL instructions (Drain/NoOp) with
# more than one sync wait. Tile's final drain carries one wait per live
# processor. Split them across single-wait SP nops; SP program order makes
# this equivalent.
def _patched_drain_and_barrier(self, tick_clock, wait_clock):
    gc = tick_clock.global_clock
    for scope, vclock in ScopedClock({None: gc}).items():
        n = len(vclock)
        for i in range(n):
            if vclock[i] > 0:
                vec = [0] * n
                vec[i] = vclock[i]
                nop_inst = self.nc.sync.nop(nofuse=True, hint="split_drain_wait")
                wait_clock.add_sem_waits(
                    nop_inst.ins, ScopedClock({scope: VectorClock(vec)})
                )
    self.nc.sync.drain()
    self.nc.all_engine_barrier()
    assert self.sems is not None
    popped = self.nc._tile_sem_poison_stack.pop()
    assert popped is self._sem_poison
    self.nc.clear_and_free_semaphores(list(self.sems.allocated().values()))
    self.nc.all_engine_barrier()


tile.TileContext._drain_and_barrier = _patched_drain_and_barrier


# This walrus build allows at most ONE sync wait on ANY instruction. Tile's
# wait-assignment emits up to 4. Post-process the serialized BIR: move all but
# the last wait of each instruction onto same-engine NoOps placed just before
# it (engine program order preserves semantics; for DMAs this gates descriptor
# submission, which is strictly more conservative).
def _split_multi_waits(m: dict) -> dict:
    for fn in m["functions"]:
        for bb in fn["blocks"]:
            out = []
            for ins in bb["instructions"]:
                si = ins.get("sync_info") or {}
                waits = si.get("on_wait") or []
                if len(waits) > 1:
                    for i, w in enumerate(waits[:-1]):
                        out.append(
                            {
                                "debug": ins.get("debug", 0),
                                "engine": ins["engine"],
                                "ins": [],
                                "outs": [],
                                "name": f"{ins['name']}-w{i}",
                                "opcode": "NoOp",
                                "sync_info": {"on_update": [], "on_wait": [w]},
                                "text_hint": "split_wait",
                            }
                        )
                    si["on_wait"] = [waits[-1]]
                out.append(ins)
            bb["instructions"] = out
    return m


_orig_to_json_bytes = bass.Bass.to_json_bytes


def _patched_to_json_bytes(self):
    import orjson

    m = orjson.loads(_orig_to_json_bytes(self))
    return orjson.dumps(_split_multi_waits(m))


bass.Bass.to_json_bytes = _patched_to_json_bytes
# ---------------------------------------------------------------------------


def build_fast_program(b_c, d_in, d_out):
    """fp8 DoubleRow fast path (gamma == 1, any beta/bias). fp16 x, fp16 out.

    Scheduling shape (Tile list-schedules greedily by emission order, so the
    structure removes timing couplings instead of relying on order):
    - mean/c are broadcast on-chip: pack per-btile columns into [P,4] ->
      PE transpose -> [1,NB] row -> ones-matmul -> [P,NB]; no DRAM roundtrip.
    - epilogue is split so PSUM drains never wait on the c scale: ACT moves
      psum*beta to fp16 immediately (sums are <2048 so fp16 is exact), DVE
      applies *c and +bias*beta afterwards from a deep fp16 buffer pool.
    - matmuls run bc-major (all 32 ogs of chunk 0, then chunk 1) so chunk-1
      signs have ~110us of slack; weight pairs stream twice (HBM has slack).
    - max/min use a fp16 tensor_tensor tree (2x rate) + short reduce.
    """
    KT = d_in // P  # contraction tiles (32)
    OG = d_out // P  # output-feature tiles (32)
    NB = 512  # matmul moving free dim = one PSUM bank of fp32
    BC = b_c // NB  # batch chunks (2)
    TPC = NB // P  # batch tiles per chunk (4)
    SC = 512  # bn_stats hardware max free size
    G = 8  # k-tiles per transposed-input DMA
    NPAIR = OG // 2

    f32 = mybir.dt.float32
    f16 = mybir.dt.float16
    fp8 = mybir.dt.float8e4
    X = mybir.AxisListType.X
    A = mybir.AluOpType
    AF = mybir.ActivationFunctionType

    nc = bass.Bass("TRN2", target_bir_lowering=False, debug=False)
    x16 = nc.dram_tensor("x16", [b_c, d_in], f16, kind="ExternalInput")
    # host-prechunked transpose: xTc[h, p, kt, j] = x[h*NB + j, kt*128 + p]
    xTc = nc.dram_tensor("xTc", [BC, P, KT, NB], f16, kind="ExternalInput")
    # host-pretiled weight signs: wS[og, p, kt, oc] = sign(W[og*128+oc, kt*128+p])
    wS = nc.dram_tensor("wS", [OG, P, KT, P], fp8, kind="ExternalInput")
    bias = nc.dram_tensor("bias", [d_out], f32, kind="ExternalInput")
    beta = nc.dram_tensor("beta", [d_out], f32, kind="ExternalInput")
    outT = nc.dram_tensor("outT", [d_out, b_c], f16, kind="ExternalOutput")
    mean_ds = [nc.dram_tensor(f"mean_d{h}", [NB], f16) for h in range(BC)]
    c_ds = [nc.dram_tensor(f"c_d{h}", [NB], f16) for h in range(BC)]

    with tile.TileContext(nc) as tc, ExitStack() as ctx:
        consts = ctx.enter_context(tc.tile_pool(name="consts", bufs=1))
        xs_p = ctx.enter_context(tc.tile_pool(name="xs", bufs=6))
        small_p = ctx.enter_context(tc.tile_pool(name="small", bufs=4))
        mh_p = ctx.enter_context(tc.tile_pool(name="mh", bufs=2))
        a_p = ctx.enter_context(tc.tile_pool(name="a", bufs=1))
        xt_p = ctx.enter_context(tc.tile_pool(name="xt", bufs=3))
        w_p = ctx.enter_context(tc.tile_pool(name="w", bufs=6))
        ep_p = ctx.enter_context(tc.tile_pool(name="ep", bufs=4))
        ps_p = ctx.enter_context(tc.tile_pool(name="ps", bufs=7, space="PSUM"))

        # --- constants ---------------------------------------------------
        eps_t = consts.tile([P, 1], f32)
        nc.vector.memset(eps_t, EPS)
        bias_t = consts.tile([P, OG], f32)
        nc.sync.dma_start(
            out=bias_t, in_=bass.AP(tensor=bias, offset=0, ap=[[1, P], [P, OG]])
        )
        beta_t = consts.tile([P, OG], f32)
        nc.sync.dma_start(
            out=beta_t, in_=bass.AP(tensor=beta, offset=0, ap=[[1, P], [P, OG]])
        )
        bb_t = consts.tile([P, OG], f32)
        nc.vector.tensor_mul(bb_t, bias_t, beta_t)

        a_t = a_p.tile([P, KT, b_c], fp8)
        HS = d_in // 2

        def emit_x_loads(h):
            tiles = []
            for bth in range(TPC):
                bt = h * TPC + bth
                xt_ = xs_p.tile([P, d_in], f16, tag="xs", name=f"xs{bt}")
                for q in range(2):
                    nc.sync.dma_start(
                        out=xt_[:, q * HS : (q + 1) * HS],
                        in_=x16[bt * P : (bt + 1) * P, q * HS : (q + 1) * HS],
                    )
                tiles.append(xt_)
            return tiles

        def emit_xtg_loads(h, gis):
            out = {}
            for gi in gis:
                xtg = xt_p.tile([P, G, NB], f16, tag="xtg", name=f"xtg{h}_{gi}")
                nc.sync.dma_start(
                    out=xtg,
                    in_=bass.AP(
                        tensor=xTc,
                        offset=h * P * KT * NB + gi * G * NB,
                        ap=[[KT * NB, P], [1, G * NB]],
                    ),
                )
                out[gi] = xtg
            return out

        def emit_w_pair(bc, pr):
            wt = w_p.tile([P, 2, KT, P], fp8, tag="w", name=f"wp{bc}_{pr}")
            nc.sync.dma_start(
                out=wt,
                in_=bass.AP(
                    tensor=wS,
                    offset=pr * 2 * P * KT * P,
                    ap=[[KT * P, P], [P * KT * P, 2], [1, KT * P]],
                ),
            )
            return wt

        def emit_bcast(col4, ds, out_name, out_dtype):
            """[P, TPC] fp16 per-btile columns -> one batched store to DRAM ->
            one partition-stride-0 broadcast load [P, NB]."""
            nc.sync.dma_start(
                out=bass.AP(tensor=ds, offset=0, ap=[[1, P], [P, TPC]]),
                in_=col4,
            )
            out = consts.tile([P, NB], out_dtype, name=out_name)
            nc.sync.dma_start(
                out=out, in_=bass.AP(tensor=ds, offset=0, ap=[[0, P], [1, NB]])
            )
            return out

        def emit_stats(h, x_tiles):
            """bn mean/var per btile; means packed into m4 columns."""
            m4 = consts.tile([P, TPC], f16, name=f"m4_{h}")
            mvs = []
            for bth in range(TPC):
                xt_ = x_tiles[bth]
                xr = xt_.rearrange("p (n f) -> p n f", f=SC)
                st = small_p.tile([P, d_in // SC, 6], f32, tag="bnst")
                for i in range(d_in // SC):
                    nc.vector.bn_stats(out=st[:, i, :], in_=xr[:, i, :])
                mv = small_p.tile([P, 2], f32, tag="mv", name=f"mv{h}_{bth}")
                nc.vector.bn_aggr(out=mv, in_=st)
                nc.scalar.activation(
                    out=m4[:, bth : bth + 1], in_=mv[:, 0:1], func=AF.Copy
                )
                mvs.append(mv)
            return m4, mvs

        def emit_signs(h, mean_b, xtg_pre):
            for gi in range(KT // G):
                if gi in xtg_pre:
                    xtg = xtg_pre[gi]
                else:
                    xtg = emit_xtg_loads(h, [gi])[gi]
                for r in range(G):
                    kt = gi * G + r
                    nc.vector.tensor_sub(xtg[:, r, :], xtg[:, r, :], mean_b)
                    nc.scalar.sign(
                        out=a_t[:, kt, h * NB : (h + 1) * NB], in_=xtg[:, r, :]
                    )

        def tree_reduce(xt_, op, nm):
            h1 = mh_p.tile([P, d_in // 2], f16, tag="mh1", name=f"h1{nm}")
            nc.vector.tensor_tensor(
                out=h1, in0=xt_[:, : d_in // 2], in1=xt_[:, d_in // 2 :], op=op
            )
            h2 = mh_p.tile([P, d_in // 4], f16, tag="mh2", name=f"h2{nm}")
            nc.vector.tensor_tensor(
                out=h2, in0=h1[:, : d_in // 4], in1=h1[:, d_in // 4 :], op=op
            )
            h3 = mh_p.tile([P, d_in // 8], f16, tag="mh3", name=f"h3{nm}")
            nc.vector.tensor_tensor(
                out=h3, in0=h2[:, : d_in // 8], in1=h2[:, d_in // 8 :], op=op
            )
            r = small_p.tile([P, 1], f32, tag=f"r{nm}")
            nc.vector.tensor_reduce(out=r, in_=h3, axis=X, op=op)
            return r

        def emit_cscale(h, x_tiles, mvs):
            """c = max(max-mean, mean-min) * rsqrt(var+eps) per btile, packed
            into c4 columns; tree max/min run at the fp16 2x element rate."""
            c4 = consts.tile([P, TPC], f16, name=f"c4_{h}")
            for bth in range(TPC):
                xt_ = x_tiles[bth]
                mv = mvs[bth]
                mx = tree_reduce(xt_, A.max, f"x{h}_{bth}")
                mn = tree_reduce(xt_, A.min, f"n{h}_{bth}")
                t1 = small_p.tile([P, 1], f32, tag="t1")
                nc.vector.tensor_scalar(
                    out=t1, in0=mx, scalar1=mv[:, 0:1], scalar2=None, op0=A.subtract
                )
                t2 = small_p.tile([P, 1], f32, tag="t2")
                nc.vector.tensor_sub(t2, mv[:, 0:1], mn)
                amax = small_p.tile([P, 1], f32, tag="amax")
                nc.vector.tensor_max(amax, t1, t2)
                std = small_p.tile([P, 1], f32, tag="std")
                nc.scalar.activation(
                    out=std, in_=mv[:, 1:2], func=AF.Sqrt, bias=eps_t
                )
                rstd = small_p.tile([P, 1], f32, tag="rstd")
                nc.vector.reciprocal(rstd, std)
                nc.vector.tensor_mul(c4[:, bth : bth + 1], amax, rstd)
            return emit_bcast(c4, c_ds[h], f"cb{h}", f16)

        # --- prologue: chunk-0 inputs + first transposed groups + first
        # weight pairs own the DMA queue heads.
        x0_tiles = emit_x_loads(0)
        xtg0_pre = emit_xtg_loads(0, range(2))
        w_tiles = {(0, pr): emit_w_pair(0, pr) for pr in range(4)}

        m4_0, mvs0 = emit_stats(0, x0_tiles)
        mean_b0 = emit_bcast(m4_0, mean_ds[0], "mean_b0", f16)
        emit_signs(0, mean_b0, xtg0_pre)
        x1_tiles = emit_x_loads(1)
        xtg1_pre = emit_xtg_loads(1, range(2))
        cb0 = emit_cscale(0, x0_tiles, mvs0)
        m4_1, mvs1 = emit_stats(1, x1_tiles)
        mean_b1 = emit_bcast(m4_1, mean_ds[1], "mean_b1", f16)
        emit_signs(1, mean_b1, xtg1_pre)
        cb1 = emit_cscale(1, x1_tiles, mvs1)
        cbs = [cb0, cb1]

        # --- matmul + epilogue, bc-major. The ACT drain (psum*beta -> fp16)
        # frees each PSUM bank immediately; the c scale and bias run on DVE
        # from the deep t2 pool whenever cb lands.
        for bc in range(BC):
            for og in range(OG):
                pr, half = og // 2, og % 2
                if (bc, pr) not in w_tiles:
                    w_tiles[(bc, pr)] = emit_w_pair(bc, pr)
                wt = w_tiles[(bc, pr)]
                psum = ps_p.tile([P, NB], f32, tag="ps", name=f"ps{og}_{bc}")
                for g in range(KT // 2):
                    nc.tensor.matmul(
                        psum,
                        lhsT=wt[:, half, 2 * g : 2 * g + 2, :],
                        rhs=a_t[:, 2 * g : 2 * g + 2, bc * NB : (bc + 1) * NB],
                        start=(g == 0),
                        stop=(g == KT // 2 - 1),
                        perf_mode=mybir.MatmulPerfMode.DoubleRow,
                    )
                t2 = ep_p.tile([P, NB], f16, tag="t2", bufs=10)
                nc.scalar.activation(
                    out=t2, in_=psum, func=AF.Identity,
                    scale=beta_t[:, og : og + 1],
                )
                t3 = ep_p.tile([P, NB], f16, tag="t3")
                nc.vector.tensor_tensor(out=t3, in0=t2, in1=cbs[bc], op=A.mult)
                o16 = ep_p.tile([P, NB], f16, tag="o16")
                nc.vector.tensor_scalar(
                    out=o16, in0=t3, scalar1=bb_t[:, og : og + 1],
                    scalar2=None, op0=A.add,
                )
                nc.sync.dma_start(
                    out=outT[og * P : (og + 1) * P, bc * NB : (bc + 1) * NB],
                    in_=o16,
                )

    return nc


def build_general_program(b_c, d_in, d_out):
    """bf16 fallback for gamma != 1: signs scaled by 1/gamma, no DoubleRow."""
    KT = d_in // P
    OG = d_out // P
    NB = 512
    BC = b_c // NB
    SC = min(512, d_in)
    nstat = d_in // SC

    f32 = mybir.dt.float32
    bf16 = mybir.dt.bfloat16
    X = mybir.AxisListType
    A = mybir.AluOpType
    AF = mybir.ActivationFunctionType
    G = min(4, KT)

    nc = bass.Bass("TRN2", target_bir_lowering=False, debug=False)
    x = nc.dram_tensor("x", [b_c, d_in], f32, kind="ExternalInput")
    xTc = nc.dram_tensor("xTc", [BC, P, KT, NB], f32, kind="ExternalInput")
    w4 = nc.dram_tensor("w4", [OG, P, KT, P], bf16, kind="ExternalInput")
    bias = nc.dram_tensor("bias", [d_out], f32, kind="ExternalInput")
    beta = nc.dram_tensor("beta", [d_out], f32, kind="ExternalInput")
    gamma = nc.dram_tensor("gamma", [d_in], f32, kind="ExternalInput")
    outT = nc.dram_tensor("outT", [d_out, b_c], f32, kind="ExternalOutput")
    mean_ds = [nc.dram_tensor(f"mean_d{h}", [NB], f32) for h in range(BC)]
    c_ds = [nc.dram_tensor(f"c_d{h}", [NB], f32) for h in range(BC)]

    XS = 1024
    NQ = d_in // XS

    with tile.TileContext(nc) as tc, ExitStack() as ctx:
        consts = ctx.enter_context(tc.tile_pool(name="consts", bufs=1))
        xs_p = ctx.enter_context(tc.tile_pool(name="xs", bufs=6))
        small_p = ctx.enter_context(tc.tile_pool(name="small", bufs=4))
        a_p = ctx.enter_context(tc.tile_pool(name="a", bufs=1))
        xt_p = ctx.enter_context(tc.tile_pool(name="xt", bufs=2))
        w_p = ctx.enter_context(tc.tile_pool(name="w", bufs=3))
        sw_p = ctx.enter_context(tc.tile_pool(name="sw", bufs=2))
        ep_p = ctx.enter_context(tc.tile_pool(name="ep", bufs=4))
        ps_p = ctx.enter_context(tc.tile_pool(name="ps", bufs=2 * BC, space="PSUM"))

        eps_t = consts.tile([P, 1], f32)
        nc.vector.memset(eps_t, EPS)
        bias_t = consts.tile([P, OG], f32)
        nc.sync.dma_start(
            out=bias_t, in_=bass.AP(tensor=bias, offset=0, ap=[[1, P], [P, OG]])
        )
        beta_t = consts.tile([P, OG], f32)
        nc.sync.dma_start(
            out=beta_t, in_=bass.AP(tensor=beta, offset=0, ap=[[1, P], [P, OG]])
        )
        bb_t = consts.tile([P, OG], f32)
        nc.vector.tensor_mul(bb_t, bias_t, beta_t)
        gamma_t = consts.tile([P, KT], f32)
        nc.sync.dma_start(
            out=gamma_t, in_=bass.AP(tensor=gamma, offset=0, ap=[[1, P], [P, KT]])
        )
        invg = consts.tile([P, KT], f32)
        nc.vector.reciprocal(invg, gamma_t)

        a_t = a_p.tile([P, KT, b_c], bf16)
        TPC = NB // P
        mean_bs = []
        cbs = []
        for h in range(BC):
            for bth in range(TPC):
                bt = h * TPC + bth
                st = small_p.tile([P, nstat, 6], f32, tag="bnst")
                mx4 = small_p.tile([P, NQ], f32, tag="mx4", name=f"mx{bth}")
                mn4 = small_p.tile([P, NQ], f32, tag="mn4", name=f"mn{bth}")
                for q in range(NQ):
                    xt_ = xs_p.tile([P, XS], f32, tag="xs", name=f"xs{bt}_{q}")
                    nc.sync.dma_start(
                        out=xt_,
                        in_=x[bt * P : (bt + 1) * P, q * XS : (q + 1) * XS],
                    )
                    xr = xt_.rearrange("p (n f) -> p n f", f=SC)
                    for i in range(XS // SC):
                        nc.vector.bn_stats(
                            out=st[:, q * (XS // SC) + i, :], in_=xr[:, i, :]
                        )
                    nc.vector.tensor_reduce(
                        out=mx4[:, q : q + 1], in_=xt_, axis=X.X, op=A.max
                    )
                    nc.vector.tensor_reduce(
                        out=mn4[:, q : q + 1], in_=xt_, axis=X.X, op=A.min
                    )
                mv = small_p.tile([P, 2], f32, tag="mv", name=f"mv{bth}")
                nc.vector.bn_aggr(out=mv, in_=st)
                nc.sync.dma_start(
                    out=mean_ds[h][bth * P : (bth + 1) * P], in_=mv[:, 0:1]
                )
                mx = small_p.tile([P, 1], f32, tag="mx1")
                nc.vector.tensor_reduce(out=mx, in_=mx4, axis=X.X, op=A.max)
                mn = small_p.tile([P, 1], f32, tag="mn1")
                nc.vector.tensor_reduce(out=mn, in_=mn4, axis=X.X, op=A.min)
                t1 = small_p.tile([P, 1], f32, tag="t1")
                nc.vector.tensor_scalar(
                    out=t1, in0=mx, scalar1=mv[:, 0:1], scalar2=None, op0=A.subtract
                )
                t2 = small_p.tile([P, 1], f32, tag="t2")
                nc.vector.tensor_sub(t2, mv[:, 0:1], mn)
                amax = small_p.tile([P, 1], f32, tag="amax")
                nc.vector.tensor_max(amax, t1, t2)
                std = small_p.tile([P, 1], f32, tag="std")
                nc.scalar.activation(out=std, in_=mv[:, 1:2], func=AF.Sqrt, bias=eps_t)
                rstd = small_p.tile([P, 1], f32, tag="rstd")
                nc.vector.reciprocal(rstd, std)
                cv = small_p.tile([P, 1], f32, tag="cv")
                nc.vector.tensor_mul(cv, amax, rstd)
                nc.sync.dma_start(out=c_ds[h][bth * P : (bth + 1) * P], in_=cv)

            mean_b = consts.tile([P, NB], f32, name=f"mean_b{h}")
            nc.sync.dma_start(
                out=mean_b,
                in_=bass.AP(tensor=mean_ds[h], offset=0, ap=[[0, P], [1, NB]]),
            )
            mean_bs.append(mean_b)
            cb = consts.tile([P, NB], f32, name=f"cb{h}")
            nc.sync.dma_start(
                out=cb, in_=bass.AP(tensor=c_ds[h], offset=0, ap=[[0, P], [1, NB]])
            )
            cbs.append(cb)

            for gi in range(KT // G):
                xtg = xt_p.tile([P, G, NB], f32, tag="xtg")
                nc.sync.dma_start(
                    out=xtg,
                    in_=bass.AP(
                        tensor=xTc,
                        offset=h * P * KT * NB + gi * G * NB,
                        ap=[[KT * NB, P], [1, G * NB]],
                    ),
                )
                for r in range(G):
                    kt = gi * G + r
                    nc.vector.tensor_sub(xtg[:, r, :], xtg[:, r, :], mean_b)
                    stmp = xt_p.tile([P, NB], bf16, tag="stmp")
                    nc.scalar.sign(out=stmp, in_=xtg[:, r, :])
                    nc.vector.tensor_scalar_mul(
                        out=a_t[:, kt, h * NB : (h + 1) * NB],
                        in0=stmp,
                        scalar1=invg[:, kt : kt + 1],
                    )

        for og in range(OG):
            wcol = w_p.tile([P, KT * P], bf16, tag="wcol")
            nc.sync.dma_start(
                out=wcol,
                in_=bass.AP(
                    tensor=w4, offset=og * P * KT * P, ap=[[KT * P, P], [1, KT * P]]
                ),
            )
            wcol3 = wcol.rearrange("p (kt oc) -> p kt oc", oc=P)
            sw = sw_p.tile([P, KT, P], bf16, tag="sw")
            nc.scalar.sign(out=sw, in_=wcol3)
            psums = [
                ps_p.tile([P, NB], f32, tag=f"ps{bc}", name=f"psum{bc}")
                for bc in range(BC)
            ]
            for bc in range(BC):
                for kt in range(KT):
                    nc.tensor.matmul(
                        psums[bc],
                        lhsT=sw[:, kt, :],
                        rhs=a_t[:, kt, bc * NB : (bc + 1) * NB],
                        start=(kt == 0),
                        stop=(kt == KT - 1),
                    )
            for bc in range(BC):
                t1 = ep_p.tile([P, NB], f32, tag="t1")
                nc.vector.tensor_tensor(out=t1, in0=psums[bc], in1=cbs[bc], op=A.mult)
                o_sb = ep_p.tile([P, NB], f32, tag="osb")
                nc.scalar.activation(
                    out=o_sb,
                    in_=t1,
                    func=AF.Identity,
                    bias=bb_t[:, og : og + 1],
                    scale=beta_t[:, og : og + 1],
                )
                nc.sync.dma_start(
                    out=outT[og * P : (og + 1) * P, bc * NB : (bc + 1) * NB],
                    in_=o_sb,
                )

    return nc


def kernel(input, weight, bias, gamma, beta, _run_kwargs=None):
    import ml_dtypes

    input = np.ascontiguousarray(np.asarray(input, dtype=np.float32))
    weight = np.ascontiguousarray(np.asarray(weight, dtype=np.float32))
    bias = np.ascontiguousarray(np.asarray(bias, dtype=np.float32))
    gamma = np.ascontiguousarray(np.asarray(gamma, dtype=np.float32))
    beta = np.ascontiguousarray(np.asarray(beta, dtype=np.float32))

    B, d_in = input.shape
    d_out = weight.shape[0]
    assert B % N_CORES == 0
    b_c = B // N_CORES
    OG, KT = d_out // 128, d_in // 128
    NB = 512
    BC = b_c // NB

    fast = bool(np.all(gamma == 1.0))

    if fast:
        nc = build_fast_program(b_c, d_in, d_out)
        fp8np = mybir.dt.np(mybir.dt.float8e4)
        # wS[og, p, kt, oc] = sign(W[og*128+oc, kt*128+p]), exact in fp8e4
        wS = np.ascontiguousarray(
            np.sign(weight).reshape(OG, 128, KT, 128).transpose(0, 3, 2, 1)
        ).astype(fp8np)
        x16_full = input.astype(np.float16)
        in_maps = []
        for c in range(N_CORES):
            x_c = x16_full[c * b_c : (c + 1) * b_c, :]
            xTc = np.ascontiguousarray(
                x_c.reshape(BC, NB, KT, 128).transpose(0, 3, 2, 1)
            )
            in_maps.append(
                {
                    "x16": np.ascontiguousarray(x_c),
                    "xTc": xTc,
                    "wS": wS,
                    "bias": bias,
                    "beta": beta,
                }
            )
    else:
        nc = build_general_program(b_c, d_in, d_out)
        w4 = np.ascontiguousarray(
            weight.reshape(OG, 128, KT, 128).transpose(0, 3, 2, 1)
        ).astype(ml_dtypes.bfloat16)
        in_maps = []
        for c in range(N_CORES):
            x_c = np.ascontiguousarray(input[c * b_c : (c + 1) * b_c, :])
            xTc = np.ascontiguousarray(
                x_c.reshape(BC, NB, KT, 128).transpose(0, 3, 2, 1)
            )
            in_maps.append(
                {
                    "x": x_c,
                    "xTc": xTc,
                    "w4": w4,
                    "bias": bias,
                    "beta": beta,
                    "gamma": gamma,
                }
            )

    res = run_bass_kernel_spmd(
        nc, in_maps, core_ids=list(range(N_CORES)), **(_run_kwargs or {})
    )

    out = np.empty((B, d_out), dtype=np.float32)
    for c in range(N_CORES):
        out[c * b_c : (c + 1) * b_c, :] = res.results[c]["outT"].T.astype(np.float32)
    if _run_kwargs:
        kernel.last_results = res
    return out


# revision 13
# speedup vs baseline: 1.4985x; 1.1167x over previous
"""BitLinear (layernorm -> absmax sign-quant -> sign-weight matmul -> bias*beta)
for Trainium2, batch-sharded across 8 NeuronCores.

Math (per row b, feature i, output o):
    mean_b  = mean(x[b,:]);  var_b = var(x[b,:])
    c_b     = max_i |x[b,i] - mean_b| * rsqrt(var_b + eps)
    A[b,i]  = sign(x[b,i] - mean_b)           (sign(xn) == sign(x - mean))
    out[b,o]= (c_b * sum_i A[b,i]*sign(W[o,i]) + bias[o]) * beta[o]

Fast path (gamma == 1): weight signs are computed on the host and shipped as
fp8e4 (+-1 exact), x is shipped as fp16 in both natural and transposed
layouts (sign flips from fp16 rounding cost ~6e-3 rel err, under the 2e-2
gate), the big GEMM runs fp8 DoubleRow (2 MACs/cell/cycle), absmax comes from
streaming max/min of raw x (amax = max(max-mean, mean-min)), and the output
is stored fp16 and upcast on the host. Each core handles 1024 batch rows; no
collectives. The general path (gamma != 1) keeps the slower bf16 pipeline.
"""
import sys

sys.path.insert(0, "/opt/trn_rl_repo")

from contextlib import ExitStack

import numpy as np

import concourse.bass as bass
import concourse.tile as tile
from concourse import masks, mybir
from concourse.bass_utils import run_bass_kernel_spmd
from concourse.vector_clock import ScopedClock, VectorClock

N_CORES = 8
EPS = 1e-5
P = 128


# ---------------------------------------------------------------------------
# Workaround: this walrus build rejects CTRL instructions (Drain/NoOp) with
# more than one sync wait. Tile's final drain carries one wait per live
# processor. Split them across single-wait SP nops; SP program order makes
# this equivalent.
def _patched_drain_and_barrier(self, tick_clock, wait_clock):
    gc = tick_clock.global_clock
    for scope, vclock in ScopedClock({None: gc}).items():
        n = len(vclock)
        for i in range(n):
            if vclock[i] > 0:
                vec = [0] * n
                vec[i] = vclock[i]
                nop_inst = self.nc.sync.nop(nofuse=True, hint="split_drain_wait")
                wait_clock.add_sem_waits(
                    nop_inst.ins, ScopedClock({scope: VectorClock(vec)})
                )
    self.nc.sync.drain()
    self.nc.all_engine_barrier()
    assert self.sems is not None
    popped = self.nc._tile_sem_poison_stack.pop()
    assert popped is self._sem_poison
    self.nc.clear_and_free_semaphores(list(self.sems.allocated().values()))
    self.nc.all_engine_barrier()


tile.TileContext._drain_and_barrier = _patched_drain_and_barrier


# This walrus build allows at most ONE sync wait on ANY instruction. Tile's
# wait-assignment emits up to 4. Post-process the serialized BIR: move all but
# the last wait of each instruction onto same-engine NoOps placed just before
# it (engine program order preserves semantics; for DMAs this gates descriptor
# submission, which is strictly more conservative).
def _split_multi_waits(m: dict) -> dict:
    for fn in m["functions"]:
        for bb in fn["blocks"]:
            out = []
            for ins in bb["instructions"]:
                si = ins.get("sync_info") or {}
                waits = si.get("on_wait") or []
                if len(waits) > 1:
                    for i, w in enumerate(waits[:-1]):
                        out.append(
                            {
                                "debug": ins.get("debug", 0),
                                "engine": ins["engine"],
                                "ins": [],
                                "outs": [],
                                "name": f"{ins['name']}-w{i}",
                                "opcode": "NoOp",
                                "sync_info": {"on_update": [], "on_wait": [w]},
                                "text_hint": "split_wait",
                            }
                        )
                    si["on_wait"] = [waits[-1]]
                out.append(ins)
            bb["instructions"] = out
    return m


_orig_to_json_bytes = bass.Bass.to_json_bytes


def _patched_to_json_bytes(self):
    import orjson

    m = orjson.loads(_orig_to_json_bytes(self))
    return orjson.dumps(_split_multi_waits(m))


bass.Bass.to_json_bytes = _patched_to_json_bytes
# ---------------------------------------------------------------------------


def build_fast_program(b_c, d_in, d_out):
    """fp8 DoubleRow fast path (gamma == 1, any beta/bias). fp16 x, fp16 out.

    Tile list-schedules greedily from a ready-heap by emission priority, so
    the structure removes timing couplings instead of fighting the order:
    - the sign-path mean row is computed on the PE (ones-matvec over the raw
      transposed tiles, fp32 psum) and broadcast with a second ones-matmul;
      no DRAM roundtrip, no dependence on the DVE bn chain.
    - the epilogue is split so PSUM drains never wait on the c scale: ACT
      moves psum*beta to fp16 immediately (|sums| < 2048 so fp16 is exact),
      then the idle GpSimd applies *c and +bias*beta from a deep fp16 pool.
    - matmuls run bc-major (all 32 ogs of chunk 0, then chunk 1) so chunk-1
      signs have ~110us of slack; weight pairs stream twice (HBM has slack).
    - c = max(max-mean, mean-min)*rsqrt(var+eps) uses fp16 tensor_tensor
      trees (2x rate); chunk-1's c block is emitted last so the scheduler
      slots it into DVE idle time, self-timed by the cb1 data dependency.
    - DMA discipline: every dma_start dispatches serially on the sync engine
      (~0.6us each, 8 slots) and descriptors cost ~155ns regardless of size,
      so transfers are merged into >=4KB-per-partition runs.
    """
    KT = d_in // P  # contraction tiles (32)
    OG = d_out // P  # output-feature tiles (32)
    NB = 512  # matmul moving free dim = one PSUM bank of fp32
    BC = b_c // NB  # batch chunks (2)
    TPC = NB // P  # batch tiles per chunk (4)
    SC = 512  # bn_stats hardware max free size
    G = 8  # k-tiles per transposed-input DMA

    f32 = mybir.dt.float32
    f16 = mybir.dt.float16
    fp8 = mybir.dt.float8e4
    X = mybir.AxisListType.X
    A = mybir.AluOpType
    AF = mybir.ActivationFunctionType

    nc = bass.Bass("TRN2", target_bir_lowering=False, debug=False)
    x16 = nc.dram_tensor("x16", [b_c, d_in], f16, kind="ExternalInput")
    # host-prechunked transpose: xTc[h, p, kt, j] = x[h*NB + j, kt*128 + p]
    xTc = nc.dram_tensor("xTc", [BC, P, KT, NB], f16, kind="ExternalInput")
    # host-pretiled weight signs: wS[og, p, kt, oc] = sign(W[og*128+oc, kt*128+p])
    wS = nc.dram_tensor("wS", [OG, P, KT, P], fp8, kind="ExternalInput")
    bias = nc.dram_tensor("bias", [d_out], f32, kind="ExternalInput")
    beta = nc.dram_tensor("beta", [d_out], f32, kind="ExternalInput")
    outT = nc.dram_tensor("outT", [d_out, b_c], f16, kind="ExternalOutput")
    c_ds = [nc.dram_tensor(f"c_d{h}", [NB], f16) for h in range(BC)]

    with tile.TileContext(nc) as tc, ExitStack() as ctx:
        consts = ctx.enter_context(tc.tile_pool(name="consts", bufs=1))
        xs_p = ctx.enter_context(tc.tile_pool(name="xs", bufs=6))
        small_p = ctx.enter_context(tc.tile_pool(name="small", bufs=4))
        mh_p = ctx.enter_context(tc.tile_pool(name="mh", bufs=2))
        a_p = ctx.enter_context(tc.tile_pool(name="a", bufs=1))
        xt_p = ctx.enter_context(tc.tile_pool(name="xt", bufs=4))
        w_p = ctx.enter_context(tc.tile_pool(name="w", bufs=6))
        ep_p = ctx.enter_context(tc.tile_pool(name="ep", bufs=4))
        ps_p = ctx.enter_context(tc.tile_pool(name="ps", bufs=6, space="PSUM"))
        psb_p = ctx.enter_context(tc.tile_pool(name="psb", bufs=2, space="PSUM"))

        # --- constants ---------------------------------------------------
        eps_t = consts.tile([P, 1], f32)
        nc.vector.memset(eps_t, EPS)
        bias_t = consts.tile([P, OG], f32)
        nc.sync.dma_start(
            out=bias_t, in_=bass.AP(tensor=bias, offset=0, ap=[[1, P], [P, OG]])
        )
        beta_t = consts.tile([P, OG], f32)
        nc.sync.dma_start(
            out=beta_t, in_=bass.AP(tensor=beta, offset=0, ap=[[1, P], [P, OG]])
        )
        bb_t = consts.tile([P, OG], f32)
        nc.vector.tensor_mul(bb_t, bias_t, beta_t)
        ones_k = consts.tile([P, 1], f16)
        nc.vector.memset(ones_k, 1.0)
        ones_r = consts.tile([1, P], f16)
        nc.vector.memset(ones_r, 1.0)

        a_t = a_p.tile([P, KT, b_c], fp8)
        HS = d_in // 2

        def emit_x_loads(h):
            tiles = []
            for bth in range(TPC):
                bt = h * TPC + bth
                xt_ = xs_p.tile([P, d_in], f16, tag="xs", name=f"xs{bt}")
                for q in range(2):
                    nc.sync.dma_start(
                        out=xt_[:, q * HS : (q + 1) * HS],
                        in_=x16[bt * P : (bt + 1) * P, q * HS : (q + 1) * HS],
                    )
                tiles.append(xt_)
            return tiles

        def emit_xtg_loads(h):
            out = []
            for gi in range(KT // G):
                xtg = xt_p.tile([P, G, NB], f16, tag="xtg", name=f"xtg{h}_{gi}")
                nc.sync.dma_start(
                    out=xtg,
                    in_=bass.AP(
                        tensor=xTc,
                        offset=h * P * KT * NB + gi * G * NB,
                        ap=[[KT * NB, P], [1, G * NB]],
                    ),
                )
                out.append(xtg)
            return out

        def emit_w_pair(bc, pr):
            wt = w_p.tile([P, 2, KT, P], fp8, tag="w", name=f"wp{bc}_{pr}")
            nc.sync.dma_start(
                out=wt,
                in_=bass.AP(
                    tensor=wS,
                    offset=pr * 2 * P * KT * P,
                    ap=[[KT * P, P], [P * KT * P, 2], [1, KT * P]],
                ),
            )
            return wt

        def emit_mean_row(h, xtg_tiles):
            """mean over features as a [1, NB] row: PE ones-matvec over the
            raw transposed tiles (fp32 psum accumulate), then a second
            ones-matmul broadcasts it across partitions. No DRAM roundtrip."""
            pm = psb_p.tile([P, NB], f32, tag="psb", name=f"pm{h}")
            for gi in range(KT // G):
                for r in range(G):
                    kt = gi * G + r
                    nc.tensor.matmul(
                        pm[0:1, :],
                        lhsT=ones_k,
                        rhs=xtg_tiles[gi][:, r, :],
                        start=(kt == 0),
                        stop=(kt == KT - 1),
                    )
            mrow = consts.tile([1, NB], f16, name=f"mrow{h}")
            nc.scalar.activation(
                out=mrow, in_=pm[0:1, :], func=AF.Copy, scale=1.0 / d_in
            )
            pb = psb_p.tile([P, NB], f32, tag="psb", name=f"pb{h}")
            nc.tensor.matmul(pb, lhsT=ones_r, rhs=mrow, start=True, stop=True)
            mean_b = consts.tile([P, NB], f16, name=f"mean_b{h}")
            nc.scalar.activation(out=mean_b, in_=pb, func=AF.Copy)
            return mean_b

        def emit_stats(h, x_tiles):
            """bn mean/var per btile (feeds the c scale only)."""
            mvs = []
            for bth in range(TPC):
                xt_ = x_tiles[bth]
                xr = xt_.rearrange("p (n f) -> p n f", f=SC)
                st = small_p.tile([P, d_in // SC, 6], f32, tag="bnst")
                for i in range(d_in // SC):
                    nc.vector.bn_stats(out=st[:, i, :], in_=xr[:, i, :])
                mv = small_p.tile([P, 2], f32, tag="mv", name=f"mv{h}_{bth}")
                nc.vector.bn_aggr(out=mv, in_=st)
                mvs.append(mv)
            return mvs

        def emit_signs(h, mean_b, xtg_tiles):
            for gi in range(KT // G):
                xtg = xtg_tiles[gi]
                for r in range(G):
                    kt = gi * G + r
                    nc.vector.tensor_sub(xtg[:, r, :], xtg[:, r, :], mean_b)
                    nc.scalar.sign(
                        out=a_t[:, kt, h * NB : (h + 1) * NB], in_=xtg[:, r, :]
                    )

        def tree_reduce(xt_, op, nm):
            h1 = mh_p.tile([P, d_in // 2], f16, tag="mh1", name=f"h1{nm}")
            nc.vector.tensor_tensor(
                out=h1, in0=xt_[:, : d_in // 2], in1=xt_[:, d_in // 2 :], op=op
            )
            h2 = mh_p.tile([P, d_in // 4], f16, tag="mh2", name=f"h2{nm}")
            nc.vector.tensor_tensor(
                out=h2, in0=h1[:, : d_in // 4], in1=h1[:, d_in // 4 :], op=op
            )
            h3 = mh_p.tile([P, d_in // 8], f16, tag="mh3", name=f"h3{nm}")
            nc.vector.tensor_tensor(
                out=h3, in0=h2[:, : d_in // 8], in1=h2[:, d_in // 8 :], op=op
            )
            r = small_p.tile([P, 1], f32, tag=f"r{nm}")
            nc.vector.tensor_reduce(out=r, in_=h3, axis=X, op=op)
            return r

        def emit_cscale(h, x_tiles, mvs):
            """c = max(max-mean, mean-min) * rsqrt(var+eps) per btile, packed
            into c4 columns; one batched store + pstride-0 broadcast."""
            c4 = consts.tile([P, TPC], f16, name=f"c4_{h}")
            for bth in range(TPC):
                xt_ = x_tiles[bth]
                mv = mvs[bth]
                mx = tree_reduce(xt_, A.max, f"x{h}_{bth}")
                mn = tree_reduce(xt_, A.min, f"n{h}_{bth}")
                t1 = small_p.tile([P, 1], f32, tag="t1")
                nc.vector.tensor_scalar(
                    out=t1, in0=mx, scalar1=mv[:, 0:1], scalar2=None, op0=A.subtract
                )
                t2 = small_p.tile([P, 1], f32, tag="t2")
                nc.vector.tensor_sub(t2, mv[:, 0:1], mn)
                amax = small_p.tile([P, 1], f32, tag="amax")
                nc.vector.tensor_max(amax, t1, t2)
                std = small_p.tile([P, 1], f32, tag="std")
                nc.scalar.activation(
                    out=std, in_=mv[:, 1:2], func=AF.Sqrt, bias=eps_t
                )
                rstd = small_p.tile([P, 1], f32, tag="rstd")
                nc.vector.reciprocal(rstd, std)
                nc.vector.tensor_mul(c4[:, bth : bth + 1], amax, rstd)
            nc.sync.dma_start(
                out=bass.AP(tensor=c_ds[h], offset=0, ap=[[1, P], [P, TPC]]),
                in_=c4,
            )
            cb = consts.tile([P, NB], f16, name=f"cb{h}")
            nc.sync.dma_start(
                out=cb, in_=bass.AP(tensor=c_ds[h], offset=0, ap=[[0, P], [1, NB]])
            )
            return cb

        # --- prologue: chunk-0 inputs + all its transposed groups + first
        # weight pairs own the DMA queue heads.
        x0_tiles = emit_x_loads(0)
        xtg0 = emit_xtg_loads(0)
        w_tiles = {(0, pr): emit_w_pair(0, pr) for pr in range(4)}

        mean_b0 = emit_mean_row(0, xtg0)
        mvs0 = emit_stats(0, x0_tiles)
        emit_signs(0, mean_b0, xtg0)
        x1_tiles = emit_x_loads(1)
        xtg1 = emit_xtg_loads(1)
        cb0 = emit_cscale(0, x0_tiles, mvs0)
        mean_b1 = emit_mean_row(1, xtg1)
        mvs1 = emit_stats(1, x1_tiles)
        emit_signs(1, mean_b1, xtg1)
        cbs = [cb0, None]

        # --- matmul + epilogue, bc-major. ACT drains each psum immediately
        # (psum*beta -> fp16); GpSimd applies the c scale and bias afterwards
        # so neither PE nor the drains ever wait on cb.
        for bc in range(BC):
            for og in range(OG):
                pr, half = og // 2, og % 2
                if (bc, pr) not in w_tiles:
                    w_tiles[(bc, pr)] = emit_w_pair(bc, pr)
                wt = w_tiles[(bc, pr)]
                psum = ps_p.tile([P, NB], f32, tag="ps", name=f"ps{og}_{bc}")
                for g in range(KT // 2):
                    nc.tensor.matmul(
                        psum,
                        lhsT=wt[:, half, 2 * g : 2 * g + 2, :],
                        rhs=a_t[:, 2 * g : 2 * g + 2, bc * NB : (bc + 1) * NB],
                        start=(g == 0),
                        stop=(g == KT // 2 - 1),
                        perf_mode=mybir.MatmulPerfMode.DoubleRow,
                    )
                t2 = ep_p.tile([P, NB], f16, tag="t2", bufs=12)
                nc.scalar.activation(
                    out=t2, in_=psum, func=AF.Identity,
                    scale=beta_t[:, og : og + 1],
                )
                if bc == 1 and og == 0:
                    # chunk-1 c block, emitted here so the scheduler slots it
                    # into DVE idle time right before the bc1 epilogues.
                    cbs[1] = emit_cscale(1, x1_tiles, mvs1)
                t3 = ep_p.tile([P, NB], f16, tag="t3")
                nc.gpsimd.tensor_mul(t3, t2, cbs[bc])
                o16 = ep_p.tile([P, NB], f16, tag="o16")
                nc.vector.tensor_scalar(
                    out=o16, in0=t3, scalar1=bb_t[:, og : og + 1],
                    scalar2=None, op0=A.add,
                )
                nc.sync.dma_start(
                    out=outT[og * P : (og + 1) * P, bc * NB : (bc + 1) * NB],
                    in_=o16,
                )

    return nc


def build_general_program(b_c, d_in, d_out):
    """bf16 fallback for gamma != 1: signs scaled by 1/gamma, no DoubleRow."""
    KT = d_in // P
    OG = d_out // P
    NB = 512
    BC = b_c // NB
    SC = min(512, d_in)
    nstat = d_in // SC

    f32 = mybir.dt.float32
    bf16 = mybir.dt.bfloat16
    X = mybir.AxisListType
    A = mybir.AluOpType
    AF = mybir.ActivationFunctionType
    G = min(4, KT)

    nc = bass.Bass("TRN2", target_bir_lowering=False, debug=False)
    x = nc.dram_tensor("x", [b_c, d_in], f32, kind="ExternalInput")
    xTc = nc.dram_tensor("xTc", [BC, P, KT, NB], f32, kind="ExternalInput")
    w4 = nc.dram_tensor("w4", [OG, P, KT, P], bf16, kind="ExternalInput")
    bias = nc.dram_tensor("bias", [d_out], f32, kind="ExternalInput")
    beta = nc.dram_tensor("beta", [d_out], f32, kind="ExternalInput")
    gamma = nc.dram_tensor("gamma", [d_in], f32, kind="ExternalInput")
    outT = nc.dram_tensor("outT", [d_out, b_c], f32, kind="ExternalOutput")
    mean_ds = [nc.dram_tensor(f"mean_d{h}", [NB], f32) for h in range(BC)]
    c_ds = [nc.dram_tensor(f"c_d{h}", [NB], f32) for h in range(BC)]

    XS = 1024
    NQ = d_in // XS

    with tile.TileContext(nc) as tc, ExitStack() as ctx:
        consts = ctx.enter_context(tc.tile_pool(name="consts", bufs=1))
        xs_p = ctx.enter_context(tc.tile_pool(name="xs", bufs=6))
        small_p = ctx.enter_context(tc.tile_pool(name="small", bufs=4))
        a_p = ctx.enter_context(tc.tile_pool(name="a", bufs=1))
        xt_p = ctx.enter_context(tc.tile_pool(name="xt", bufs=2))
        w_p = ctx.enter_context(tc.tile_pool(name="w", bufs=3))
        sw_p = ctx.enter_context(tc.tile_pool(name="sw", bufs=2))
        ep_p = ctx.enter_context(tc.tile_pool(name="ep", bufs=4))
        ps_p = ctx.enter_context(tc.tile_pool(name="ps", bufs=2 * BC, space="PSUM"))

        eps_t = consts.tile([P, 1], f32)
        nc.vector.memset(eps_t, EPS)
        bias_t = consts.tile([P, OG], f32)
        nc.sync.dma_start(
            out=bias_t, in_=bass.AP(tensor=bias, offset=0, ap=[[1, P], [P, OG]])
        )
        beta_t = consts.tile([P, OG], f32)
        nc.sync.dma_start(
            out=beta_t, in_=bass.AP(tensor=beta, offset=0, ap=[[1, P], [P, OG]])
        )
        bb_t = consts.tile([P, OG], f32)
        nc.vector.tensor_mul(bb_t, bias_t, beta_t)
        gamma_t = consts.tile([P, KT], f32)
        nc.sync.dma_start(
            out=gamma_t, in_=bass.AP(tensor=gamma, offset=0, ap=[[1, P], [P, KT]])
        )
        invg = consts.tile([P, KT], f32)
        nc.vector.reciprocal(invg, gamma_t)

        a_t = a_p.tile([P, KT, b_c], bf16)
        TPC = NB // P
        mean_bs = []
        cbs = []
        for h in range(BC):
            for bth in range(TPC):
                bt = h * TPC + bth
                st = small_p.tile([P, nstat, 6], f32, tag="bnst")
                mx4 = small_p.tile([P, NQ], f32, tag="mx4", name=f"mx{bth}")
                mn4 = small_p.tile([P, NQ], f32, tag="mn4", name=f"mn{bth}")
                for q in range(NQ):
                    xt_ = xs_p.tile([P, XS], f32, tag="xs", name=f"xs{bt}_{q}")
                    nc.sync.dma_start(
                        out=xt_,
                        in_=x[bt * P : (bt + 1) * P, q * XS : (q + 1) * XS],
                    )
                    xr = xt_.rearrange("p (n f) -> p n f", f=SC)
                    for i in range(XS // SC):
                        nc.vector.bn_stats(
                            out=st[:, q * (XS // SC) + i, :], in_=xr[:, i, :]
                        )
                    nc.vector.tensor_reduce(
                        out=mx4[:, q : q + 1], in_=xt_, axis=X.X, op=A.max
                    )
                    nc.vector.tensor_reduce(
                        out=mn4[:, q : q + 1], in_=xt_, axis=X.X, op=A.min
                    )
                mv = small_p.tile([P, 2], f32, tag="mv", name=f"mv{bth}")
                nc.vector.bn_aggr(out=mv, in_=st)
                nc.sync.dma_start(
                    out=mean_ds[h][bth * P : (bth + 1) * P], in_=mv[:, 0:1]
                )
                mx = small_p.tile([P, 1], f32, tag="mx1")
                nc.vector.tensor_reduce(out=mx, in_=mx4, axis=X.X, op=A.max)
                mn = small_p.tile([P, 1], f32, tag="mn1")
                nc.vector.tensor_reduce(out=mn, in_=mn4, axis=X.X, op=A.min)
                t1 = small_p.tile([P, 1], f32, tag="t1")
                nc.vector.tensor_scalar(
                    out=t1, in0=mx, scalar1=mv[:, 0:1], scalar2=None, op0=A.subtract
                )
                t2 = small_p.tile([P, 1], f32, tag="t2")
                nc.vector.tensor_sub(t2, mv[:, 0:1], mn)
                amax = small_p.tile([P, 1], f32, tag="amax")
                nc.vector.tensor_max(amax, t1, t2)
                std = small_p.tile([P, 1], f32, tag="std")
                nc.scalar.activation(out=std, in_=mv[:, 1:2], func=AF.Sqrt, bias=eps_t)
                rstd = small_p.tile([P, 1], f32, tag="rstd")
                nc.vector.reciprocal(rstd, std)
                cv = small_p.tile([P, 1], f32, tag="cv")
                nc.vector.tensor_mul(cv, amax, rstd)
                nc.sync.dma_start(out=c_ds[h][bth * P : (bth + 1) * P], in_=cv)

            mean_b = consts.tile([P, NB], f32, name=f"mean_b{h}")
            nc.sync.dma_start(
                out=mean_b,
                in_=bass.AP(tensor=mean_ds[h], offset=0, ap=[[0, P], [1, NB]]),
            )
            mean_bs.append(mean_b)
            cb = consts.tile([P, NB], f32, name=f"cb{h}")
            nc.sync.dma_start(
                out=cb, in_=bass.AP(tensor=c_ds[h], offset=0, ap=[[0, P], [1, NB]])
            )
            cbs.append(cb)

            for gi in range(KT // G):
                xtg = xt_p.tile([P, G, NB], f32, tag="xtg")
                nc.sync.dma_start(
                    out=xtg,
                    in_=bass.AP(
                        tensor=xTc,
                        offset=h * P * KT * NB + gi * G * NB,
                        ap=[[KT * NB, P], [1, G * NB]],
                    ),
                )
                for r in range(G):
                    kt = gi * G + r
                    nc.vector.tensor_sub(xtg[:, r, :], xtg[:, r, :], mean_b)
                    stmp = xt_p.tile([P, NB], bf16, tag="stmp")
                    nc.scalar.sign(out=stmp, in_=xtg[:, r, :])
                    nc.vector.tensor_scalar_mul(
                        out=a_t[:, kt, h * NB : (h + 1) * NB],
                        in0=stmp,
                        scalar1=invg[:, kt : kt + 1],
                    )

        for og in range(OG):
            wcol = w_p.tile([P, KT * P], bf16, tag="wcol")
            nc.sync.dma_start(
                out=wcol,
                in_=bass.AP(
                    tensor=w4, offset=og * P * KT * P, ap=[[KT * P, P], [1, KT * P]]
                ),
            )
            wcol3 = wcol.rearrange("p (kt oc) -> p kt oc", oc=P)
            sw = sw_p.tile([P, KT, P], bf16, tag="sw")
            nc.scalar.sign(out=sw, in_=wcol3)
            psums = [
                ps_p.tile([P, NB], f32, tag=f"ps{bc}", name=f"psum{bc}")
                for bc in range(BC)
            ]
            for bc in range(BC):
                for kt in range(KT):
                    nc.tensor.matmul(
                        psums[bc],
                        lhsT=sw[:, kt, :],
                        rhs=a_t[:, kt, bc * NB : (bc + 1) * NB],
                        start=(kt == 0),
                        stop=(kt == KT - 1),
                    )
            for bc in range(BC):
                t1 = ep_p.tile([P, NB], f32, tag="t1")
                nc.vector.tensor_tensor(out=t1, in0=psums[bc], in1=cbs[bc], op=A.mult)
                o_sb = ep_p.tile([P, NB], f32, tag="osb")
                nc.scalar.activation(
                    out=o_sb,
                    in_=t1,
                    func=AF.Identity,
                    bias=bb_t[:, og : og + 1],
                    scale=beta_t[:, og : og + 1],
                )
                nc.sync.dma_start(
                    out=outT[og * P : (og + 1) * P, bc * NB : (bc + 1) * NB],
                    in_=o_sb,
                )

    return nc


def kernel(input, weight, bias, gamma, beta, _run_kwargs=None):
    import ml_dtypes

    input = np.ascontiguousarray(np.asarray(input, dtype=np.float32))
    weight = np.ascontiguousarray(np.asarray(weight, dtype=np.float32))
    bias = np.ascontiguousarray(np.asarray(bias, dtype=np.float32))
    gamma = np.ascontiguousarray(np.asarray(gamma, dtype=np.float32))
    beta = np.ascontiguousarray(np.asarray(beta, dtype=np.float32))

    B, d_in = input.shape
    d_out = weight.shape[0]
    assert B % N_CORES == 0
    b_c = B // N_CORES
    OG, KT = d_out // 128, d_in // 128
    NB = 512
    BC = b_c // NB

    fast = bool(np.all(gamma == 1.0))

    if fast:
        nc = build_fast_program(b_c, d_in, d_out)
        fp8np = mybir.dt.np(mybir.dt.float8e4)
        # wS[og, p, kt, oc] = sign(W[og*128+oc, kt*128+p]), exact in fp8e4
        wS = np.ascontiguousarray(
            np.sign(weight).reshape(OG, 128, KT, 128).transpose(0, 3, 2, 1)
        ).astype(fp8np)
        x16_full = input.astype(np.float16)
        in_maps = []
        for c in range(N_CORES):
            x_c = x16_full[c * b_c : (c + 1) * b_c, :]
            xTc = np.ascontiguousarray(
                x_c.reshape(BC, NB, KT, 128).transpose(0, 3, 2, 1)
            )
            in_maps.append(
                {
                    "x16": np.ascontiguousarray(x_c),
                    "xTc": xTc,
                    "wS": wS,
                    "bias": bias,
                    "beta": beta,
                }
            )
    else:
        nc = build_general_program(b_c, d_in, d_out)
        w4 = np.ascontiguousarray(
            weight.reshape(OG, 128, KT, 128).transpose(0, 3, 2, 1)
        ).astype(ml_dtypes.bfloat16)
        in_maps = []
        for c in range(N_CORES):
            x_c = np.ascontiguousarray(input[c * b_c : (c + 1) * b_c, :])
            xTc = np.ascontiguousarray(
                x_c.reshape(BC, NB, KT, 128).transpose(0, 3, 2, 1)
            )
            in_maps.append(
                {
                    "x": x_c,
                    "xTc": xTc,
                    "w4": w4,
                    "bias": bias,
                    "beta": beta,
                    "gamma": gamma,
                }
            )

    res = run_bass_kernel_spmd(
        nc, in_maps, core_ids=list(range(N_CORES)), **(_run_kwargs or {})
    )

    out = np.empty((B, d_out), dtype=np.float32)
    for c in range(N_CORES):
        out[c * b_c : (c + 1) * b_c, :] = res.results[c]["outT"].T.astype(np.float32)
    if _run_kwargs:
        kernel.last_results = res
    return out
